# revision 12
# baseline (speedup 1.0000x reference)
"""Causal self-attention with RoPE on 8 Trainium2 NeuronCores.

Sharding: Megatron-style head parallelism. 16 heads / 8 cores = 2 heads per
core. Each core computes q/k/v projections for its 2 heads (column-parallel),
full causal attention for those heads, and a partial output projection
(row-parallel slice of w_o). The host sums the 8 partial outputs.

v2 changes vs the f32r baseline:
- All matmul operands and all HBM traffic are bf16 (fp32 PSUM accumulate).
  Halves DMA bytes and SBUF read pressure; rel-err budget ~0.8% << 2e-2.
- Softmax denominators accumulate via an all-ones [128,128] lhsT, so the
  per-q sums land already replicated across all 128 partitions: the old
  [1,TQ] sum + ones-column broadcast matmul (which ran at 2 cyc/row) and
  the PSUM->SBUF staging copies are gone. The reciprocal runs directly on
  the PSUM tile via reciprocal_approx_fast (~5x faster than reciprocal),
  and the normalization multiply reads the PV PSUM tile directly.
- Fine-grained causal diagonal: the TQ x TQ diagonal square of each q-group
  is processed in 128-wide q-subchunks, only the lower-triangular kv tiles
  are computed, and the single exact-diagonal tile per subchunk is masked
  multiplicatively on the DVE after exp (zero the j<r triangle) instead of
  accumulating a -1e30 additive mask through the PE. Saves ~25% of the
  attention-phase PE rows.

On-chip layout: everything transposed. Host passes xT = x^T per batch
[B, D, T]; projections produce qT/kT [dh, t] directly and v [t, dh]
(lhsT = xT chunk, rhs = w_v slice). Scores are computed transposed,
ST[kv, q] = matmul(lhsT=kT_chunk, rhs=qT_group), which makes P^T directly
usable as the moving operand of the PV matmul - no on-chip transposes.
Normalization + output projection for each q group are emitted one q group
late so the PE stream never waits on the DVE reciprocal. No max-subtraction:
logits are q.k/sqrt(dh) with unit-ish variance, |logit| < ~8 << 88 (fp32 exp
overflow), identical math to the max-subtracted reference. The attention
scale 1/sqrt(dh) is folded into w_q on the host.
"""

import numpy as np

B, T, D = 4, 2048, 2048
H, DH = 16, 128
NCORES = 8
HPC = H // NCORES  # heads per core
THETA = 10000.0

TT = 512  # projection t-tile (moving dim of q/k projection matmuls)
TQ = 512  # attention q-group width
TK = 128  # kv tile (contraction chunk of PV / partition dim of ST)
WG = 64.0  # weight prescale: keeps fp8(w*WG) clear of the e4m3 subnormals
RS = 16.0  # residual upscale inside the fp8 hi/lo split


def _rope_tables(seq_len, d_head, theta):
    # Matches reference.rope_cos_sin numerics, then transposes to [dh, t]
    # and folds the rotate-half sign into sin.
    inv_freq = 1.0 / (theta ** (np.arange(0, d_head, 2, dtype=np.float32) / d_head))
    t = np.arange(seq_len, dtype=np.float32)
    freqs = np.einsum("i,j->ij", t, inv_freq)
    emb = np.concatenate([freqs, freqs], axis=-1)  # [T, dh]
    cosT = np.ascontiguousarray(np.cos(emb).astype(np.float32).T)  # [dh, T]
    sinT = np.ascontiguousarray(np.sin(emb).astype(np.float32).T)
    sgn = np.ones((d_head, 1), np.float32)
    sgn[: d_head // 2] = -1.0
    return cosT, sinT * sgn


def _legalize_waits(nc, mybir):
    """Walrus on this toolchain refuses more than one embedded sync wait
    per engine instruction. Hoist extra waits into standalone
    EventSemaphore instructions on the same engine queue (the sequencer
    executes them in-stream before the instruction, same gating)."""
    n = 0
    for f in nc.m.functions:
        for bb in f.blocks:
            out = []
            for inst in bb.instructions:
                si = inst.sync_info
                if (si and si.on_wait and len(si.on_wait) > 1
                        and not isinstance(inst, mybir.InstEventSemaphore)):
                    for w in si.on_wait[:-1]:
                        out.append(mybir.InstEventSemaphore(
                            name=f"WH-{n}", engine=inst.engine,
                            sync_info=mybir.SyncInfo(
                                on_wait=[w], on_update=[])))
                        n += 1
                    inst.sync_info = mybir.SyncInfo(
                        on_wait=[si.on_wait[-1]],
                        on_update=list(si.on_update))
                out.append(inst)
            bb.instructions = out
    return n


def _build_nc(b_sz, t_sz, d_sz, legalize=True):
    import concourse.bass as bass
    import concourse.tile as tile
    from concourse import mybir

    f32 = mybir.dt.float32
    bf16 = mybir.dt.bfloat16
    f8 = mybir.dt.float8e4
    EXP = mybir.ActivationFunctionType.Exp
    LN = mybir.ActivationFunctionType.Ln
    DR = mybir.MatmulPerfMode.DoubleRow

    DC = d_sz // 128         # contraction chunks
    NQG = t_sz // TQ         # q groups per (batch, head)
    NKT = t_sz // TK         # kv tiles
    KPG = TQ // TK           # kv tiles per q group (diagonal span)

    nc = bass.Bass("TRN2", target_bir_lowering=False, debug=False,
                   enable_asserts=False, dynamic_dma_scratch_size=2048)

    NW = HPC * DH
    # fp8 DoubleRow operands: "m" tensors interleave (hi, lo) pairs along a
    # slot axis s (contracted together with the partition axis by the PE's
    # DoubleRow mode); "l" tensors hold the weight residuals, sliced in
    # consecutive-chunk pairs for the correction pass.
    xm = nc.dram_tensor("xm", [b_sz, 128, DC, 2, t_sz], f8,
                        kind="ExternalInput")
    wqm = nc.dram_tensor("wqm", [128, DC, 2, NW], f8, kind="ExternalInput")
    wkm = nc.dram_tensor("wkm", [128, DC, 2, NW], f8, kind="ExternalInput")
    wvm = nc.dram_tensor("wvm", [128, DC, 2, NW], f8, kind="ExternalInput")
    wql = nc.dram_tensor("wql", [128, DC, NW], f8, kind="ExternalInput")
    wkl = nc.dram_tensor("wkl", [128, DC, NW], f8, kind="ExternalInput")
    wvl = nc.dram_tensor("wvl", [128, DC, NW], f8, kind="ExternalInput")
    wo = nc.dram_tensor("wo", [HPC * DH, d_sz], bf16, kind="ExternalInput")
    cos = nc.dram_tensor("cos", [DH, t_sz], f32, kind="ExternalInput")
    sin = nc.dram_tensor("sin", [DH, t_sz], f32, kind="ExternalInput")
    tri = nc.dram_tensor("tri", [TK, TK], bf16, kind="ExternalInput")
    one = nc.dram_tensor("one", [128, 128], bf16, kind="ExternalInput")
    y = nc.dram_tensor("y", [b_sz, t_sz, d_sz], bf16, kind="ExternalOutput")

    xm_r = xm.ap()
    wo_r = wo.ap().rearrange("(h p) n -> p h n", p=128)
    y_r = y.ap()

    with tile.TileContext(nc) as tc:
        with (
            tc.tile_pool(name="consts", bufs=1) as consts,
            tc.tile_pool(name="wpool", bufs=1) as wpool,
            tc.tile_pool(name="qkv", bufs=1) as qkv,
            tc.tile_pool(name="xpool", bufs=3) as xpool,
            tc.tile_pool(name="rope", bufs=2) as rope,
            tc.tile_pool(name="pex", bufs=3) as pexp,
            tc.tile_pool(name="sax", bufs=2) as sax,
            tc.tile_pool(name="otn", bufs=8) as otnp,
            tc.tile_pool(name="psS", bufs=2, space="PSUM") as psS,
            tc.tile_pool(name="psO", bufs=2, space="PSUM") as psO,
            tc.tile_pool(name="psR", bufs=2, space="PSUM") as psR,
            tc.tile_pool(name="psY", bufs=2, space="PSUM") as psY,
        ):
            cos_sb = consts.tile([DH, t_sz], f32)
            sin_sb = consts.tile([DH, t_sz], f32)
            tri_sb = consts.tile([TK, TK], bf16)
            ones_sb = consts.tile([128, 128], bf16)

            wqm_sb = wpool.tile([128, DC, 2, NW], f8)
            wkm_sb = wpool.tile([128, DC, 2, NW], f8)
            wvm_sb = wpool.tile([128, DC, 2, NW], f8)
            wql_sb = wpool.tile([128, DC, NW], f8)
            wkl_sb = wpool.tile([128, DC, NW], f8)
            wvl_sb = wpool.tile([128, DC, NW], f8)
            wo_sb = wpool.tile([128, HPC, d_sz], bf16)

            # first-needed data first: the first x tile and q/k/v weight
            # chunks feed the very first matmuls, so their DMAs go at the
            # head of every queue
            xt_first = xpool.tile([128, DC, 2, TT], f8, tag="xt",
                                  name="xt_first")
            for dc in range(DC):
                nc.sync.dma_start(xt_first[:, dc, :, :],
                                  xm_r[0, :, dc, :, 0:TT])
                nc.sync.dma_start(wqm_sb[:, dc, :, :], wqm.ap()[:, dc, :, :])
                nc.sync.dma_start(wkm_sb[:, dc, :, :], wkm.ap()[:, dc, :, :])
                nc.sync.dma_start(wvm_sb[:, dc, :, :], wvm.ap()[:, dc, :, :])
            for dc in range(DC):
                nc.sync.dma_start(wql_sb[:, dc, :], wql.ap()[:, dc, :])
                nc.sync.dma_start(wkl_sb[:, dc, :], wkl.ap()[:, dc, :])
                nc.sync.dma_start(wvl_sb[:, dc, :], wvl.ap()[:, dc, :])

            def load_consts():
                # emitted after the first x tile's DMAs: nothing here is
                # needed before RoPE / attention of the first tile
                for i in range(t_sz // TT):
                    sl = slice(i * TT, (i + 1) * TT)
                    nc.sync.dma_start(cos_sb[:, sl], cos.ap()[:, sl])
                    nc.sync.dma_start(sin_sb[:, sl], sin.ap()[:, sl])
                nc.sync.dma_start(tri_sb[:], tri.ap())
                nc.sync.dma_start(ones_sb[:], one.ap())
                for hh in range(HPC):
                    for nch in range(d_sz // 512):
                        nsl = slice(nch * 512, (nch + 1) * 512)
                        nc.sync.dma_start(wo_sb[:, hh, nsl],
                                          wo_r[:, hh, nsl])

            for b in range(b_sz):
                # ---------------- phase A: projections + RoPE ----------
                qT = [qkv.tile([DH, t_sz], bf16, tag=f"qT{h}", name=f"qT{h}")
                      for h in range(HPC)]
                kT = [qkv.tile([DH, t_sz], bf16, tag=f"kT{h}", name=f"kT{h}")
                      for h in range(HPC)]
                vv = qkv.tile([128, NKT, HPC * DH], bf16, tag="vv", name="vv")

                for tt in range(t_sz // TT):
                    tsl = slice(tt * TT, (tt + 1) * TT)
                    if b == 0 and tt == 0:
                        xt = xt_first
                        load_consts()
                    else:
                        xt = xpool.tile([128, DC, 2, TT], f8, tag="xt",
                                        name="xt")
                        for dc in range(DC):
                            nc.sync.dma_start(xt[:, dc, :, :],
                                              xm_r[b, :, dc, :, tsl])

                    for h in range(HPC):
                        hs = slice(h * DH, (h + 1) * DH)
                        for dst, wm_sb, wl_sb in (
                            (qT[h], wqm_sb, wql_sb),
                            (kT[h], wkm_sb, wkl_sb),
                        ):
                            pp = psS.tile([128, TT], f32, tag="st")
                            # main: slots (x_hi, 16*x_lo) x (w_hi, w_hi/16)
                            for dc in range(DC):
                                nc.tensor.matmul(
                                    pp[:],
                                    wm_sb[:, dc, :, hs],
                                    xt[:, dc, :, :],
                                    start=(dc == 0), stop=False,
                                    perf_mode=DR, skip_group_check=True,
                                )
                            # correction: slots = consecutive chunk pairs,
                            # x_hi lanes of xt x the w residuals
                            for d2 in range(DC // 2):
                                nc.tensor.matmul(
                                    pp[:],
                                    wl_sb[:, 2 * d2:2 * d2 + 2, hs],
                                    xt[:, 2 * d2:2 * d2 + 2, 0, :],
                                    start=False, stop=(d2 == DC // 2 - 1),
                                    perf_mode=DR, skip_group_check=True,
                                )
                            # RoPE: dst = pp*cos + swap(pp)*sin_signed
                            # (1/G weight prescale folded into cos/sin)
                            sh = rope.tile([DH, TT], f32, tag="sh")
                            nc.vector.tensor_mul(
                                sh[0:64, :], pp[64:128, :], sin_sb[0:64, tsl])
                            nc.vector.tensor_mul(
                                sh[64:128, :], pp[0:64, :], sin_sb[64:128, tsl])
                            tmp = rope.tile([DH, TT], f32, tag="tmp")
                            nc.vector.tensor_mul(tmp[:], pp[:], cos_sb[:, tsl])
                            nc.vector.tensor_add(dst[:, tsl], tmp[:], sh[:])

                    for ts2 in range(TT // TK):
                        ts3 = slice(ts2 * TK, (ts2 + 1) * TK)
                        vp = psS.tile([128, TT], f32, tag="st")
                        for dc in range(DC):
                            nc.tensor.matmul(
                                vp[:, 0:HPC * DH],
                                xt[:, dc, :, ts3],
                                wvm_sb[:, dc, :, :],
                                start=(dc == 0), stop=False,
                                perf_mode=DR, skip_group_check=True,
                            )
                        for d2 in range(DC // 2):
                            nc.tensor.matmul(
                                vp[:, 0:HPC * DH],
                                xt[:, 2 * d2:2 * d2 + 2, 0, ts3],
                                wvl_sb[:, 2 * d2:2 * d2 + 2, :],
                                start=False, stop=(d2 == DC // 2 - 1),
                                perf_mode=DR, skip_group_check=True,
                            )
                        kv_i = tt * (TT // TK) + ts2
                        # 1/G weight prescale compensated here
                        nc.scalar.mul(vv[:, kv_i, :], vp[:, 0:HPC * DH],
                                      1.0 / WG)

                # ---------------- phase B + C: attention + out proj ----
                otn_tiles = {}
                pending = []
                for h in range(HPC):
                    hs = slice(h * DH, (h + 1) * DH)
                    for qi in range(NQG):
                        outp = psO.tile([DH, TQ], f32, tag="outT")
                        denp = psR.tile([DH, TQ], f32, tag="den")

                        def qk_exp(ki, qsl, n, masked):
                            # score matmul [TK, n] + exp (+ causal mask)
                            stp = psS.tile([128, TT], f32, tag="st")
                            nc.tensor.matmul(
                                stp[:, 0:n],
                                kT[h][:, ki * TK:(ki + 1) * TK],
                                qT[h][:, qsl],
                                start=True, stop=True,
                            )
                            if masked:
                                praw = pexp.tile([TK, TK], bf16, tag="praw",
                                                 bufs=3, name="praw")
                                nc.scalar.activation(praw[:], stp[:, 0:n], EXP)
                                pex = pexp.tile([TK, TK], bf16, tag="pexd",
                                                bufs=3, name="pexd")
                                nc.vector.tensor_mul(pex[:], praw[:], tri_sb[:])
                            else:
                                pex = pexp.tile([TK, TQ], bf16, tag="pex",
                                                name="pex")
                                nc.scalar.activation(pex[:, 0:n],
                                                     stp[:, 0:n], EXP)
                            return pex

                        # off-diagonal: full-width, no masking
                        nko = qi * KPG
                        for ki in range(nko):
                            pex = qk_exp(ki, slice(qi * TQ, (qi + 1) * TQ),
                                         TQ, False)
                            nc.tensor.matmul(
                                outp[:], vv[:, ki, hs], pex[:],
                                start=(ki == 0), stop=False,
                                skip_group_check=True,
                            )
                            nc.tensor.matmul(
                                denp[:], ones_sb[:], pex[:],
                                start=(ki == 0), stop=False,
                                skip_group_check=True,
                            )
                        # diagonal square: per 128-wide q-subchunk, only
                        # lower-triangular kv tiles; exact diagonal masked
                        for jj in range(KPG):
                            q0 = qi * TQ + jj * TK
                            jsl = slice(jj * TK, (jj + 1) * TK)
                            for dg in range(jj + 1):
                                ki = qi * KPG + dg
                                pex = qk_exp(ki, slice(q0, q0 + TK), TK,
                                             dg == jj)
                                st_col = (qi == 0 and dg == 0)
                                sp_col = (dg == jj)
                                nc.tensor.matmul(
                                    outp[:, jsl], vv[:, ki, hs], pex[:, 0:TK],
                                    start=st_col, stop=sp_col,
                                    skip_group_check=True,
                                )
                                nc.tensor.matmul(
                                    denp[:, jsl], ones_sb[:], pex[:, 0:TK],
                                    start=st_col, stop=sp_col,
                                    skip_group_check=True,
                                )

                        def norm_and_project(h=h, qi=qi, outp=outp, denp=denp,
                                             b=b):
                            # deferred one q-group: runs while the PE chews
                            # on the next q-group, so the reciprocal chain
                            # never stalls the PE stream. 1/den computed as
                            # exp(-ln(den)) on the ACT engine: two table ops
                            # (~1e-3 rel err, fine for a softmax denominator)
                            # instead of the 13x-slower DVE reciprocal.
                            lnt = sax.tile([DH, TQ], f32, tag="lnt",
                                           name="lnt")
                            nc.scalar.activation(lnt[:], denp[:], LN)
                            rcp = sax.tile([DH, TQ], f32, tag="rcp",
                                           name="rcp")
                            nc.scalar.activation(rcp[:], lnt[:], EXP,
                                                 scale=-1.0)
                            otn = otnp.tile([DH, TQ], bf16, tag="otn",
                                            name="otn")
                            nc.vector.tensor_mul(otn[:], outp[:], rcp[:])
                            otn_tiles[(h, qi)] = otn
                            if h != HPC - 1:
                                return
                            for tc2 in range(TQ // TK):
                                tq0 = qi * TQ + tc2 * TK
                                for nch in range(d_sz // 512):
                                    yp = psY.tile([TK, 512], f32, tag="y",
                                                  name="yp")
                                    for hh in range(HPC):
                                        nc.tensor.matmul(
                                            yp[:],
                                            otn_tiles[(hh, qi)][
                                                :, tc2 * TK:(tc2 + 1) * TK],
                                            wo_sb[:, hh,
                                                  nch * 512:(nch + 1) * 512],
                                            start=(hh == 0),
                                            stop=(hh == HPC - 1),
                                        )
                                    ysb = pexp.tile([TK, 512], bf16, tag="ysb",
                                                    bufs=3, name="ysb")
                                    if nch % 2 == 0:
                                        nc.scalar.copy(ysb[:], yp[:])
                                    else:
                                        nc.vector.tensor_copy(ysb[:], yp[:])
                                    nc.sync.dma_start(
                                        y_r[b, tq0:tq0 + TK,
                                            nch * 512:(nch + 1) * 512],
                                        ysb[:])

                        pending.append(norm_and_project)
                        if len(pending) > 1:
                            pending.pop(0)()
                for fn in pending:
                    fn()
    if legalize:
        _legalize_waits(nc, mybir)
    return nc


_NC_CACHE = {}
LAST_RESULT = None


def _get_nc(b_sz, t_sz, d_sz):
    key = (b_sz, t_sz, d_sz)
    if key not in _NC_CACHE:
        _NC_CACHE[key] = _build_nc(b_sz, t_sz, d_sz)
    return _NC_CACHE[key]


def _split_hilo(a, f8):
    """fp8 hi/lo split: a ~= hi + lo/RS with hi, lo fp8 (e4m3)."""
    hi = a.astype(f8)
    lo = ((a - hi.astype(np.float32)) * RS).astype(f8)
    return hi, lo


def _pack_w(w, f8):
    """[D, NW] f32 -> main interleaved [128, DC, 2, NW] (w_hi, w_hi/RS)
    and the UNSCALED residual fp8(w*WG - w_hi) [128, DC, NW] (it multiplies
    the x_hi lanes directly in the correction pass). All prescaled by WG."""
    d_sz, nw = w.shape
    dc = d_sz // 128
    wg = (w * WG).astype(np.float32)
    hi = wg.astype(f8)
    lo = (wg - hi.astype(np.float32)).astype(f8)
    hi_r = hi.reshape(dc, 128, nw).transpose(1, 0, 2)     # [128, DC, NW]
    lo_r = lo.reshape(dc, 128, nw).transpose(1, 0, 2)
    hi16 = (hi.astype(np.float32) / RS).astype(f8)
    hi16_r = hi16.reshape(dc, 128, nw).transpose(1, 0, 2)
    main = np.ascontiguousarray(np.stack([hi_r, hi16_r], axis=2))
    return main, np.ascontiguousarray(lo_r)


def kernel(x, w_q, w_k, w_v, w_o):
    import ml_dtypes
    from concourse.bass_utils import run_bass_kernel_spmd

    bf16 = ml_dtypes.bfloat16
    f8 = ml_dtypes.float8_e4m3
    b_sz, t_sz, d_sz = x.shape
    dc = d_sz // 128
    scale = np.float32(1.0 / np.sqrt(DH))

    xT = np.ascontiguousarray(
        np.asarray(x, np.float32).transpose(0, 2, 1))  # [B, D, T]
    x_hi, x_lo = _split_hilo(xT, f8)
    # [B, D, T] -> [B, 128, DC, T] with D = dc*128 + p
    x_hi_r = x_hi.reshape(b_sz, dc, 128, t_sz).transpose(0, 2, 1, 3)
    x_lo_r = x_lo.reshape(b_sz, dc, 128, t_sz).transpose(0, 2, 1, 3)
    xm = np.ascontiguousarray(
        np.stack([x_hi_r, x_lo_r], axis=3))  # [B, 128, DC, 2, T]

    w_q = np.asarray(w_q, np.float32)
    w_k = np.asarray(w_k, np.float32)
    w_v = np.asarray(w_v, np.float32)
    w_o = np.asarray(w_o, np.float32)
    cosT, sinT = _rope_tables(t_sz, DH, THETA)
    cosT /= np.float32(WG)  # fold away the fp8 weight prescale
    sinT /= np.float32(WG)
    r = np.arange(TK)
    tri01 = (r[None, :] >= r[:, None]).astype(bf16)  # [kv, q]: keep q >= kv

    in_maps = []
    for c in range(NCORES):
        cs = slice(c * HPC * DH, (c + 1) * HPC * DH)
        wqm, wql = _pack_w(w_q[:, cs] * scale, f8)
        wkm, wkl = _pack_w(w_k[:, cs], f8)
        wvm, wvl = _pack_w(w_v[:, cs], f8)
        in_maps.append({
            "xm": xm,
            "wqm": wqm, "wql": wql,
            "wkm": wkm, "wkl": wkl,
            "wvm": wvm, "wvl": wvl,
            "wo": np.ascontiguousarray(w_o[cs, :]).astype(bf16),
            "cos": cosT,
            "sin": sinT,
            "tri": tri01,
            "one": np.ones((128, 128), bf16),
        })

    nc = _get_nc(b_sz, t_sz, d_sz)
    res = run_bass_kernel_spmd(nc, in_maps, core_ids=list(range(NCORES)))
    global LAST_RESULT
    LAST_RESULT = res

    out = res.results[0]["y"].astype(np.float32)
    for c in range(1, NCORES):
        out += res.results[c]["y"].astype(np.float32)
    return out


# revision 21
# speedup vs baseline: 1.1048x; 1.1048x over previous
"""Causal self-attention with RoPE on 8 Trainium2 NeuronCores.

Sharding: Megatron-style head parallelism. 16 heads / 8 cores = 2 heads per
core. Each core computes q/k/v projections for its 2 heads (column-parallel),
full causal attention for those heads, and a partial output projection
(row-parallel slice of w_o). The host sums the 8 partial outputs.

v2 changes vs the f32r baseline:
- All matmul operands and all HBM traffic are bf16 (fp32 PSUM accumulate).
  Halves DMA bytes and SBUF read pressure; rel-err budget ~0.8% << 2e-2.
- Softmax denominators accumulate via an all-ones [128,128] lhsT, so the
  per-q sums land already replicated across all 128 partitions: the old
  [1,TQ] sum + ones-column broadcast matmul (which ran at 2 cyc/row) and
  the PSUM->SBUF staging copies are gone. The reciprocal runs directly on
  the PSUM tile via reciprocal_approx_fast (~5x faster than reciprocal),
  and the normalization multiply reads the PV PSUM tile directly.
- Fine-grained causal diagonal: the TQ x TQ diagonal square of each q-group
  is processed in 128-wide q-subchunks, only the lower-triangular kv tiles
  are computed, and the single exact-diagonal tile per subchunk is masked
  multiplicatively on the DVE after exp (zero the j<r triangle) instead of
  accumulating a -1e30 additive mask through the PE. Saves ~25% of the
  attention-phase PE rows.

On-chip layout: everything transposed. Host passes xT = x^T per batch
[B, D, T]; projections produce qT/kT [dh, t] directly and v [t, dh]
(lhsT = xT chunk, rhs = w_v slice). Scores are computed transposed,
ST[kv, q] = matmul(lhsT=kT_chunk, rhs=qT_group), which makes P^T directly
usable as the moving operand of the PV matmul - no on-chip transposes.
Normalization + output projection for each q group are emitted one q group
late so the PE stream never waits on the DVE reciprocal. No max-subtraction:
logits are q.k/sqrt(dh) with unit-ish variance, |logit| < ~8 << 88 (fp32 exp
overflow), identical math to the max-subtracted reference. The attention
scale 1/sqrt(dh) is folded into w_q on the host.
"""

import numpy as np

B, T, D = 4, 2048, 2048
H, DH = 16, 128
NCORES = 8
HPC = H // NCORES  # heads per core
THETA = 10000.0

TT = 512  # projection t-tile (moving dim of q/k projection matmuls)
TQ = 512  # attention q-group width
TK = 128  # kv tile (contraction chunk of PV / partition dim of ST)


def _rope_tables(seq_len, d_head, theta):
    # Matches reference.rope_cos_sin numerics, then transposes to [dh, t]
    # and folds the rotate-half sign into sin.
    inv_freq = 1.0 / (theta ** (np.arange(0, d_head, 2, dtype=np.float32) / d_head))
    t = np.arange(seq_len, dtype=np.float32)
    freqs = np.einsum("i,j->ij", t, inv_freq)
    emb = np.concatenate([freqs, freqs], axis=-1)  # [T, dh]
    cosT = np.ascontiguousarray(np.cos(emb).astype(np.float32).T)  # [dh, T]
    sinT = np.ascontiguousarray(np.sin(emb).astype(np.float32).T)
    sgn = np.ones((d_head, 1), np.float32)
    sgn[: d_head // 2] = -1.0
    return cosT, sinT * sgn


def _legalize_waits(nc, mybir):
    """Walrus on this toolchain refuses more than one embedded sync wait
    per engine instruction. Hoist extra waits into standalone
    EventSemaphore instructions on the same engine queue (the sequencer
    executes them in-stream before the instruction, same gating)."""
    n = 0
    for f in nc.m.functions:
        for bb in f.blocks:
            out = []
            for inst in bb.instructions:
                si = inst.sync_info
                if (si and si.on_wait and len(si.on_wait) > 1
                        and not isinstance(inst, mybir.InstEventSemaphore)):
                    for w in si.on_wait[:-1]:
                        out.append(mybir.InstEventSemaphore(
                            name=f"WH-{n}", engine=inst.engine,
                            sync_info=mybir.SyncInfo(
                                on_wait=[w], on_update=[])))
                        n += 1
                    inst.sync_info = mybir.SyncInfo(
                        on_wait=[si.on_wait[-1]],
                        on_update=list(si.on_update))
                out.append(inst)
            bb.instructions = out
    return n


def _build_nc(b_sz, t_sz, d_sz, legalize=True):
    import concourse.bass as bass
    import concourse.tile as tile
    from concourse import mybir

    f32 = mybir.dt.float32
    bf16 = mybir.dt.bfloat16
    EXP = mybir.ActivationFunctionType.Exp
    LN = mybir.ActivationFunctionType.Ln

    DC = d_sz // 128         # contraction chunks
    NQG = t_sz // TQ         # q groups per (batch, head)
    NKT = t_sz // TK         # kv tiles
    KPG = TQ // TK           # kv tiles per q group (diagonal span)

    nc = bass.Bass("TRN2", target_bir_lowering=False, debug=False,
                   enable_asserts=False, dynamic_dma_scratch_size=2048)

    NW = HPC * DH
    xT = nc.dram_tensor("xT", [b_sz, d_sz, t_sz], bf16, kind="ExternalInput")
    wq = nc.dram_tensor("wq", [d_sz, NW], bf16, kind="ExternalInput")
    wk = nc.dram_tensor("wk", [d_sz, NW], bf16, kind="ExternalInput")
    wv = nc.dram_tensor("wv", [d_sz, NW], bf16, kind="ExternalInput")
    wo = nc.dram_tensor("wo", [HPC * DH, d_sz], bf16, kind="ExternalInput")
    cos = nc.dram_tensor("cos", [DH, t_sz], f32, kind="ExternalInput")
    sin = nc.dram_tensor("sin", [DH, t_sz], f32, kind="ExternalInput")
    tri = nc.dram_tensor("tri", [TK, TK], bf16, kind="ExternalInput")
    one = nc.dram_tensor("one", [128, 128], bf16, kind="ExternalInput")
    y = nc.dram_tensor("y", [b_sz, t_sz, d_sz], bf16, kind="ExternalOutput")

    xT_r = xT.ap().rearrange("b (dc p) t -> b p dc t", p=128)
    wq_r = wq.ap().rearrange("(dc p) n -> p dc n", p=128)
    wk_r = wk.ap().rearrange("(dc p) n -> p dc n", p=128)
    wv_r = wv.ap().rearrange("(dc p) n -> p dc n", p=128)
    wo_r = wo.ap().rearrange("(h p) n -> p h n", p=128)
    y_r = y.ap()

    with tile.TileContext(nc) as tc:
        with (
            tc.tile_pool(name="consts", bufs=1) as consts,
            tc.tile_pool(name="wpool", bufs=1) as wpool,
            tc.tile_pool(name="qkv", bufs=1) as qkv,
            tc.tile_pool(name="xpool", bufs=3) as xpool,
            tc.tile_pool(name="rope", bufs=2) as rope,
            tc.tile_pool(name="pex", bufs=3) as pexp,
            tc.tile_pool(name="sax", bufs=2) as sax,
            tc.tile_pool(name="otn", bufs=8) as otnp,
            tc.tile_pool(name="psS", bufs=2, space="PSUM") as psS,
            tc.tile_pool(name="psO", bufs=2, space="PSUM") as psO,
            tc.tile_pool(name="psR", bufs=2, space="PSUM") as psR,
            tc.tile_pool(name="psY", bufs=2, space="PSUM") as psY,
        ):
            cos_sb = consts.tile([DH, t_sz], f32)
            sin_sb = consts.tile([DH, t_sz], f32)
            tri_sb = consts.tile([TK, TK], bf16)
            ones_sb = consts.tile([128, 128], bf16)

            wq_sb = wpool.tile([128, DC, NW], bf16)
            wk_sb = wpool.tile([128, DC, NW], bf16)
            wv_sb = wpool.tile([128, DC, NW], bf16)
            wo_sb = wpool.tile([128, HPC, d_sz], bf16)

            # first-needed data first: the first x tile and q/k/v weight
            # chunks feed the very first matmuls, so their DMAs go at the
            # head of every queue; within that, q's weights before k's
            # before v's (phase order within the first tile)
            xt_first = xpool.tile([128, DC, TT], bf16, tag="xt",
                                  name="xt_first")
            for dc in range(DC):
                nc.sync.dma_start(xt_first[:, dc, :], xT_r[0, :, dc, 0:TT])
                nc.sync.dma_start(wq_sb[:, dc, :], wq_r[:, dc, :])
            for dc in range(DC):
                nc.sync.dma_start(wk_sb[:, dc, :], wk_r[:, dc, :])
            for dc in range(DC):
                nc.sync.dma_start(wv_sb[:, dc, :], wv_r[:, dc, :])

            def load_consts():
                # emitted after the first x tile's DMAs: nothing here is
                # needed before RoPE / attention of the first tile
                for i in range(t_sz // TT):
                    sl = slice(i * TT, (i + 1) * TT)
                    nc.sync.dma_start(cos_sb[:, sl], cos.ap()[:, sl])
                    nc.sync.dma_start(sin_sb[:, sl], sin.ap()[:, sl])
                nc.sync.dma_start(tri_sb[:], tri.ap())
                nc.sync.dma_start(ones_sb[:], one.ap())
                for hh in range(HPC):
                    for nch in range(d_sz // 512):
                        nsl = slice(nch * 512, (nch + 1) * 512)
                        nc.sync.dma_start(wo_sb[:, hh, nsl],
                                          wo_r[:, hh, nsl])

            for b in range(b_sz):
                # ---------------- phase A: projections + RoPE ----------
                qT = [qkv.tile([DH, t_sz], bf16, tag=f"qT{h}", name=f"qT{h}")
                      for h in range(HPC)]
                kT = [qkv.tile([DH, t_sz], bf16, tag=f"kT{h}", name=f"kT{h}")
                      for h in range(HPC)]
                vv = qkv.tile([128, NKT, HPC * DH], bf16, tag="vv", name="vv")

                for tt in range(t_sz // TT):
                    tsl = slice(tt * TT, (tt + 1) * TT)
                    if b == 0 and tt == 0:
                        xt = xt_first
                        load_consts()
                    else:
                        xt = xpool.tile([128, DC, TT], bf16, tag="xt",
                                        name="xt")
                        for dc in range(DC):
                            nc.sync.dma_start(xt[:, dc, :],
                                              xT_r[b, :, dc, tsl])

                    for h in range(HPC):
                        hs = slice(h * DH, (h + 1) * DH)
                        for dst, w_sb in ((qT[h], wq_sb), (kT[h], wk_sb)):
                            pp = psS.tile([128, TT], f32, tag="st")
                            for dc in range(DC):
                                nc.tensor.matmul(
                                    pp[:],
                                    w_sb[:, dc, hs],
                                    xt[:, dc, :],
                                    start=(dc == 0), stop=(dc == DC - 1),
                                )
                            # RoPE: dst = pp*cos + swap(pp)*sin_signed
                            sh = rope.tile([DH, TT], f32, tag="sh")
                            nc.vector.tensor_mul(
                                sh[0:64, :], pp[64:128, :], sin_sb[0:64, tsl])
                            nc.vector.tensor_mul(
                                sh[64:128, :], pp[0:64, :], sin_sb[64:128, tsl])
                            tmp = rope.tile([DH, TT], f32, tag="tmp")
                            nc.vector.tensor_mul(tmp[:], pp[:], cos_sb[:, tsl])
                            nc.vector.tensor_add(dst[:, tsl], tmp[:], sh[:])

                    for ts2 in range(TT // TK):
                        ts3 = slice(ts2 * TK, (ts2 + 1) * TK)
                        vp = psS.tile([128, TT], f32, tag="st")
                        for dc in range(DC):
                            nc.tensor.matmul(
                                vp[:, 0:HPC * DH],
                                xt[:, dc, ts3],
                                wv_sb[:, dc, :],
                                start=(dc == 0), stop=(dc == DC - 1),
                            )
                        kv_i = tt * (TT // TK) + ts2
                        nc.scalar.copy(vv[:, kv_i, :], vp[:, 0:HPC * DH])

                # ---------------- phase B + C: attention + out proj ----
                otn_tiles = {}
                pending = []
                for h in range(HPC):
                    hs = slice(h * DH, (h + 1) * DH)
                    for qi in range(NQG):
                        outp = psO.tile([DH, TQ], f32, tag="outT")
                        denp = psR.tile([DH, TQ], f32, tag="den")

                        def qk_exp(ki, q0, n, masked):
                            # score matmul [TK, n] + exp (+ causal mask on
                            # the leading TK columns = the exact-diagonal
                            # tile, zeroed multiplicatively after exp)
                            stp = psS.tile([128, TT], f32, tag="st")
                            nc.tensor.matmul(
                                stp[:, 0:n],
                                kT[h][:, ki * TK:(ki + 1) * TK],
                                qT[h][:, q0:q0 + n],
                                start=True, stop=True,
                            )
                            pex = pexp.tile([TK, TQ], bf16, tag="pex",
                                            name="pex")
                            nc.scalar.activation(pex[:, 0:n], stp[:, 0:n],
                                                 EXP)
                            if masked:
                                nc.vector.tensor_mul(
                                    pex[:, 0:TK], pex[:, 0:TK], tri_sb[:])
                            return pex

                        # off-diagonal: full-width, no masking
                        nko = qi * KPG
                        for ki in range(nko):
                            pex = qk_exp(ki, qi * TQ, TQ, False)
                            nc.tensor.matmul(
                                outp[:], vv[:, ki, hs], pex[:],
                                start=(ki == 0), stop=False,
                                skip_group_check=True,
                            )
                            nc.tensor.matmul(
                                denp[:], ones_sb[:], pex[:],
                                start=(ki == 0), stop=False,
                                skip_group_check=True,
                            )
                        # diagonal square: kv tile dg covers the contiguous
                        # q-range [dg*TK, TQ) of this group - the causally
                        # live columns - with the exact-diagonal tile at its
                        # head. One score/PV/den matmul per dg.
                        for dg in range(KPG):
                            ki = qi * KPG + dg
                            n = TQ - dg * TK
                            dsl = slice(dg * TK, TQ)
                            pex = qk_exp(ki, qi * TQ + dg * TK, n, True)
                            st_col = (qi == 0 and dg == 0)
                            sp_col = (dg == KPG - 1)
                            nc.tensor.matmul(
                                outp[:, dsl], vv[:, ki, hs], pex[:, 0:n],
                                start=st_col, stop=sp_col,
                                skip_group_check=True,
                            )
                            nc.tensor.matmul(
                                denp[:, dsl], ones_sb[:], pex[:, 0:n],
                                start=st_col, stop=sp_col,
                                skip_group_check=True,
                            )

                        def norm_and_project(h=h, qi=qi, outp=outp, denp=denp,
                                             b=b):
                            # deferred one q-group: runs while the PE chews
                            # on the next q-group, so the reciprocal chain
                            # never stalls the PE stream. 1/den computed as
                            # exp(-ln(den)) on the ACT engine: two table ops
                            # (~1e-3 rel err, fine for a softmax denominator)
                            # instead of the 13x-slower DVE reciprocal.
                            lnt = sax.tile([DH, TQ], f32, tag="lnt",
                                           name="lnt")
                            nc.scalar.activation(lnt[:], denp[:], LN)
                            rcp = sax.tile([DH, TQ], f32, tag="rcp",
                                           name="rcp")
                            nc.scalar.activation(rcp[:], lnt[:], EXP,
                                                 scale=-1.0)
                            otn = otnp.tile([DH, TQ], bf16, tag="otn",
                                            name="otn")
                            nc.vector.tensor_mul(otn[:], outp[:], rcp[:])
                            otn_tiles[(h, qi)] = otn
                            if h != HPC - 1:
                                return
                            for tc2 in range(TQ // TK):
                                tq0 = qi * TQ + tc2 * TK
                                for nch in range(d_sz // 512):
                                    yp = psY.tile([TK, 512], f32, tag="y",
                                                  name="yp")
                                    for hh in range(HPC):
                                        nc.tensor.matmul(
                                            yp[:],
                                            otn_tiles[(hh, qi)][
                                                :, tc2 * TK:(tc2 + 1) * TK],
                                            wo_sb[:, hh,
                                                  nch * 512:(nch + 1) * 512],
                                            start=(hh == 0),
                                            stop=(hh == HPC - 1),
                                        )
                                    # DVE only: the ACT engine stays free
                                    # for the latency-critical softmax exps
                                    ysb = pexp.tile([TK, 512], bf16, tag="ysb",
                                                    bufs=3, name="ysb")
                                    nc.vector.tensor_copy(ysb[:], yp[:])
                                    nc.sync.dma_start(
                                        y_r[b, tq0:tq0 + TK,
                                            nch * 512:(nch + 1) * 512],
                                        ysb[:])

                        pending.append(norm_and_project)
                        if len(pending) > 1:
                            pending.pop(0)()
                for fn in pending:
                    fn()
    if legalize:
        _legalize_waits(nc, mybir)
    return nc


_NC_CACHE = {}
LAST_RESULT = None


def _get_nc(b_sz, t_sz, d_sz):
    key = (b_sz, t_sz, d_sz)
    if key not in _NC_CACHE:
        _NC_CACHE[key] = _build_nc(b_sz, t_sz, d_sz)
    return _NC_CACHE[key]


def kernel(x, w_q, w_k, w_v, w_o):
    import ml_dtypes
    from concourse.bass_utils import run_bass_kernel_spmd

    bf16 = ml_dtypes.bfloat16
    b_sz, t_sz, d_sz = x.shape
    scale = np.float32(1.0 / np.sqrt(DH))

    xT = np.ascontiguousarray(
        np.asarray(x, np.float32).transpose(0, 2, 1)).astype(bf16)
    w_q = np.asarray(w_q, np.float32)
    w_k = np.asarray(w_k, np.float32)
    w_v = np.asarray(w_v, np.float32)
    w_o = np.asarray(w_o, np.float32)
    cosT, sinT = _rope_tables(t_sz, DH, THETA)
    r = np.arange(TK)
    tri01 = (r[None, :] >= r[:, None]).astype(bf16)  # [kv, q]: keep q >= kv

    in_maps = []
    for c in range(NCORES):
        cs = slice(c * HPC * DH, (c + 1) * HPC * DH)
        in_maps.append({
            "xT": xT,
            "wq": np.ascontiguousarray(w_q[:, cs] * scale).astype(bf16),
            "wk": np.ascontiguousarray(w_k[:, cs]).astype(bf16),
            "wv": np.ascontiguousarray(w_v[:, cs]).astype(bf16),
            "wo": np.ascontiguousarray(w_o[cs, :]).astype(bf16),
            "cos": cosT,
            "sin": sinT,
            "tri": tri01,
            "one": np.ones((128, 128), bf16),
        })

    nc = _get_nc(b_sz, t_sz, d_sz)
    res = run_bass_kernel_spmd(nc, in_maps, core_ids=list(range(NCORES)))
    global LAST_RESULT
    LAST_RESULT = res

    out = res.results[0]["y"].astype(np.float32)
    for c in range(1, NCORES):
        out += res.results[c]["y"].astype(np.float32)
    return out


# revision 24
# speedup vs baseline: 1.3141x; 1.1894x over previous
"""Causal self-attention with RoPE on 8 Trainium2 NeuronCores.

Sharding: Megatron-style head parallelism. 16 heads / 8 cores = 2 heads per
core. Each core computes q/k/v projections for its 2 heads (column-parallel),
full causal attention for those heads, and a partial output projection
(row-parallel slice of w_o). The host sums the 8 partial outputs.

v2 changes vs the f32r baseline:
- All matmul operands and all HBM traffic are bf16 (fp32 PSUM accumulate).
  Halves DMA bytes and SBUF read pressure; rel-err budget ~0.8% << 2e-2.
- Softmax denominators accumulate via an all-ones [128,128] lhsT, so the
  per-q sums land already replicated across all 128 partitions: the old
  [1,TQ] sum + ones-column broadcast matmul (which ran at 2 cyc/row) and
  the PSUM->SBUF staging copies are gone. The reciprocal runs directly on
  the PSUM tile via reciprocal_approx_fast (~5x faster than reciprocal),
  and the normalization multiply reads the PV PSUM tile directly.
- Fine-grained causal diagonal: the TQ x TQ diagonal square of each q-group
  is processed in 128-wide q-subchunks, only the lower-triangular kv tiles
  are computed, and the single exact-diagonal tile per subchunk is masked
  multiplicatively on the DVE after exp (zero the j<r triangle) instead of
  accumulating a -1e30 additive mask through the PE. Saves ~25% of the
  attention-phase PE rows.

On-chip layout: everything transposed. Host passes xT = x^T per batch
[B, D, T]; projections produce qT/kT [dh, t] directly and v [t, dh]
(lhsT = xT chunk, rhs = w_v slice). Scores are computed transposed,
ST[kv, q] = matmul(lhsT=kT_chunk, rhs=qT_group), which makes P^T directly
usable as the moving operand of the PV matmul - no on-chip transposes.
Normalization + output projection for each q group are emitted one q group
late so the PE stream never waits on the DVE reciprocal. No max-subtraction:
logits are q.k/sqrt(dh) with unit-ish variance, |logit| < ~8 << 88 (fp32 exp
overflow), identical math to the max-subtracted reference. The attention
scale 1/sqrt(dh) is folded into w_q on the host.
"""

import numpy as np

B, T, D = 4, 2048, 2048
H, DH = 16, 128
NCORES = 8
HPC = H // NCORES  # heads per core
THETA = 10000.0

TT = 512  # projection t-tile (moving dim of q/k projection matmuls)
TQ = 512  # attention q-group width
TK = 128  # kv tile (contraction chunk of PV / partition dim of ST)


def _rope_tables(seq_len, d_head, theta):
    # Matches reference.rope_cos_sin numerics, then transposes to [dh, t]
    # and folds the rotate-half sign into sin.
    inv_freq = 1.0 / (theta ** (np.arange(0, d_head, 2, dtype=np.float32) / d_head))
    t = np.arange(seq_len, dtype=np.float32)
    freqs = np.einsum("i,j->ij", t, inv_freq)
    emb = np.concatenate([freqs, freqs], axis=-1)  # [T, dh]
    cosT = np.ascontiguousarray(np.cos(emb).astype(np.float32).T)  # [dh, T]
    sinT = np.ascontiguousarray(np.sin(emb).astype(np.float32).T)
    sgn = np.ones((d_head, 1), np.float32)
    sgn[: d_head // 2] = -1.0
    return cosT, sinT * sgn


def _legalize_waits(nc, mybir):
    """Walrus on this toolchain refuses more than one embedded sync wait
    per engine instruction. Hoist extra waits into standalone
    EventSemaphore instructions on the same engine queue (the sequencer
    executes them in-stream before the instruction, same gating)."""
    n = 0
    for f in nc.m.functions:
        for bb in f.blocks:
            out = []
            for inst in bb.instructions:
                si = inst.sync_info
                if (si and si.on_wait and len(si.on_wait) > 1
                        and not isinstance(inst, mybir.InstEventSemaphore)):
                    for w in si.on_wait[:-1]:
                        out.append(mybir.InstEventSemaphore(
                            name=f"WH-{n}", engine=inst.engine,
                            sync_info=mybir.SyncInfo(
                                on_wait=[w], on_update=[])))
                        n += 1
                    inst.sync_info = mybir.SyncInfo(
                        on_wait=[si.on_wait[-1]],
                        on_update=list(si.on_update))
                out.append(inst)
            bb.instructions = out
    return n


def _build_nc(b_sz, t_sz, d_sz, legalize=True):
    import concourse.bass as bass
    import concourse.tile as tile
    from concourse import mybir

    f32 = mybir.dt.float32
    bf16 = mybir.dt.bfloat16
    EXP = mybir.ActivationFunctionType.Exp
    LN = mybir.ActivationFunctionType.Ln

    DC = d_sz // 128         # contraction chunks
    NQG = t_sz // TQ         # q groups per (batch, head)
    NKT = t_sz // TK         # kv tiles
    KPG = TQ // TK           # kv tiles per q group (diagonal span)

    nc = bass.Bass("TRN2", target_bir_lowering=False, debug=False,
                   enable_asserts=False, dynamic_dma_scratch_size=2048)

    NW = HPC * DH
    xT = nc.dram_tensor("xT", [b_sz, d_sz, t_sz], bf16, kind="ExternalInput")
    wq = nc.dram_tensor("wq", [d_sz, NW], bf16, kind="ExternalInput")
    wk = nc.dram_tensor("wk", [d_sz, NW], bf16, kind="ExternalInput")
    wv = nc.dram_tensor("wv", [d_sz, NW], bf16, kind="ExternalInput")
    wo = nc.dram_tensor("wo", [HPC * DH, d_sz], bf16, kind="ExternalInput")
    cos = nc.dram_tensor("cos", [DH, t_sz], f32, kind="ExternalInput")
    sin = nc.dram_tensor("sin", [DH, t_sz], f32, kind="ExternalInput")
    tri = nc.dram_tensor("tri", [TK, TK], bf16, kind="ExternalInput")
    one = nc.dram_tensor("one", [128, 128], bf16, kind="ExternalInput")
    y = nc.dram_tensor("y", [b_sz, t_sz, d_sz], bf16, kind="ExternalOutput")

    xT_r = xT.ap().rearrange("b (dc p) t -> b p dc t", p=128)
    wq_r = wq.ap().rearrange("(dc p) n -> p dc n", p=128)
    wk_r = wk.ap().rearrange("(dc p) n -> p dc n", p=128)
    wv_r = wv.ap().rearrange("(dc p) n -> p dc n", p=128)
    wo_r = wo.ap().rearrange("(h p) n -> p h n", p=128)
    y_r = y.ap()

    with tile.TileContext(nc) as tc:
        with (
            tc.tile_pool(name="consts", bufs=1) as consts,
            tc.tile_pool(name="wpool", bufs=1) as wpool,
            tc.tile_pool(name="qkv", bufs=1) as qkv,
            tc.tile_pool(name="xpool", bufs=3) as xpool,
            tc.tile_pool(name="rope", bufs=2) as rope,
            tc.tile_pool(name="pex", bufs=3) as pexp,
            tc.tile_pool(name="sax", bufs=2) as sax,
            tc.tile_pool(name="otn", bufs=8) as otnp,
            tc.tile_pool(name="psS", bufs=2, space="PSUM") as psS,
            tc.tile_pool(name="psO", bufs=2, space="PSUM") as psO,
            tc.tile_pool(name="psR", bufs=2, space="PSUM") as psR,
            tc.tile_pool(name="psY", bufs=2, space="PSUM") as psY,
        ):
            cos_sb = consts.tile([DH, t_sz], f32)
            sin_sb = consts.tile([DH, t_sz], f32)
            tri_sb = consts.tile([TK, TK], bf16)
            ones_sb = consts.tile([128, 128], bf16)

            wq_sb = wpool.tile([128, DC, NW], bf16)
            wk_sb = wpool.tile([128, DC, NW], bf16)
            wv_sb = wpool.tile([128, DC, NW], bf16)
            wo_sb = wpool.tile([128, HPC, d_sz], bf16)

            # DMA issue runs on two engine queues in parallel: the sync
            # engine paces the x-tile / y streams, the (otherwise idle)
            # gpsimd engine issues weights + constants, so the cold start
            # is not serialized on one sequencer's ~0.6us per descriptor.
            xt_first = xpool.tile([128, DC, TT], bf16, tag="xt",
                                  name="xt_first")
            for dc in range(DC):
                nc.sync.dma_start(xt_first[:, dc, :], xT_r[0, :, dc, 0:TT])
                nc.scalar.dma_start(wq_sb[:, dc, :], wq_r[:, dc, :])
            for dc in range(DC):
                nc.scalar.dma_start(wk_sb[:, dc, :], wk_r[:, dc, :])
            # cos/sin of the first tile gate the very first RoPE op
            nc.scalar.dma_start(cos_sb[:, 0:TT], cos.ap()[:, 0:TT])
            nc.scalar.dma_start(sin_sb[:, 0:TT], sin.ap()[:, 0:TT])
            for dc in range(DC):
                nc.scalar.dma_start(wv_sb[:, dc, :], wv_r[:, dc, :])

            def load_consts():
                # everything here is first needed in the attention phase
                # (t > ~90us): emitted after tile 1's x DMAs
                for i in range(1, t_sz // TT):
                    sl = slice(i * TT, (i + 1) * TT)
                    nc.scalar.dma_start(cos_sb[:, sl], cos.ap()[:, sl])
                    nc.scalar.dma_start(sin_sb[:, sl], sin.ap()[:, sl])
                nc.scalar.dma_start(tri_sb[:], tri.ap())
                nc.scalar.dma_start(ones_sb[:], one.ap())
                for hh in range(HPC):
                    for nch in range(d_sz // 512):
                        nsl = slice(nch * 512, (nch + 1) * 512)
                        nc.scalar.dma_start(wo_sb[:, hh, nsl],
                                            wo_r[:, hh, nsl])

            for b in range(b_sz):
                # ---------------- phase A: projections + RoPE ----------
                qT = [qkv.tile([DH, t_sz], bf16, tag=f"qT{h}", name=f"qT{h}")
                      for h in range(HPC)]
                kT = [qkv.tile([DH, t_sz], bf16, tag=f"kT{h}", name=f"kT{h}")
                      for h in range(HPC)]
                vv = qkv.tile([128, NKT, HPC * DH], bf16, tag="vv", name="vv")

                for tt in range(t_sz // TT):
                    tsl = slice(tt * TT, (tt + 1) * TT)
                    if b == 0 and tt == 0:
                        xt = xt_first
                    else:
                        xt = xpool.tile([128, DC, TT], bf16, tag="xt",
                                        name="xt")
                        for dc in range(DC):
                            nc.sync.dma_start(xt[:, dc, :],
                                              xT_r[b, :, dc, tsl])
                    if b == 0 and tt == 1:
                        load_consts()

                    for h in range(HPC):
                        hs = slice(h * DH, (h + 1) * DH)
                        for dst, w_sb in ((qT[h], wq_sb), (kT[h], wk_sb)):
                            pp = psS.tile([128, TT], f32, tag="st")
                            for dc in range(DC):
                                nc.tensor.matmul(
                                    pp[:],
                                    w_sb[:, dc, hs],
                                    xt[:, dc, :],
                                    start=(dc == 0), stop=(dc == DC - 1),
                                )
                            # RoPE: dst = pp*cos + swap(pp)*sin_signed
                            sh = rope.tile([DH, TT], f32, tag="sh")
                            nc.vector.tensor_mul(
                                sh[0:64, :], pp[64:128, :], sin_sb[0:64, tsl])
                            nc.vector.tensor_mul(
                                sh[64:128, :], pp[0:64, :], sin_sb[64:128, tsl])
                            tmp = rope.tile([DH, TT], f32, tag="tmp")
                            nc.vector.tensor_mul(tmp[:], pp[:], cos_sb[:, tsl])
                            nc.vector.tensor_add(dst[:, tsl], tmp[:], sh[:])

                    for ts2 in range(TT // TK):
                        ts3 = slice(ts2 * TK, (ts2 + 1) * TK)
                        vp = psS.tile([128, TT], f32, tag="st")
                        for dc in range(DC):
                            nc.tensor.matmul(
                                vp[:, 0:HPC * DH],
                                xt[:, dc, ts3],
                                wv_sb[:, dc, :],
                                start=(dc == 0), stop=(dc == DC - 1),
                            )
                        kv_i = tt * (TT // TK) + ts2
                        nc.scalar.copy(vv[:, kv_i, :], vp[:, 0:HPC * DH])

                # ---------------- phase B + C: attention + out proj ----
                otn_tiles = {}
                pending = []
                for h in range(HPC):
                    hs = slice(h * DH, (h + 1) * DH)
                    for qi in range(NQG):
                        outp = psO.tile([DH, TQ], f32, tag="outT")
                        denp = psR.tile([DH, TQ], f32, tag="den")

                        def qk_exp(ki, q0, n, masked):
                            # score matmul [TK, n] + exp (+ causal mask on
                            # the leading TK columns = the exact-diagonal
                            # tile, zeroed multiplicatively after exp)
                            stp = psS.tile([128, TT], f32, tag="st")
                            nc.tensor.matmul(
                                stp[:, 0:n],
                                kT[h][:, ki * TK:(ki + 1) * TK],
                                qT[h][:, q0:q0 + n],
                                start=True, stop=True,
                            )
                            pex = pexp.tile([TK, TQ], bf16, tag="pex",
                                            name="pex")
                            nc.scalar.activation(pex[:, 0:n], stp[:, 0:n],
                                                 EXP)
                            if masked:
                                nc.vector.tensor_mul(
                                    pex[:, 0:TK], pex[:, 0:TK], tri_sb[:])
                            return pex

                        # off-diagonal: full-width, no masking
                        nko = qi * KPG
                        for ki in range(nko):
                            pex = qk_exp(ki, qi * TQ, TQ, False)
                            nc.tensor.matmul(
                                outp[:], vv[:, ki, hs], pex[:],
                                start=(ki == 0), stop=False,
                                skip_group_check=True,
                            )
                            nc.tensor.matmul(
                                denp[:], ones_sb[:], pex[:],
                                start=(ki == 0), stop=False,
                                skip_group_check=True,
                            )
                        # diagonal square: kv tile dg covers the contiguous
                        # q-range [dg*TK, TQ) of this group - the causally
                        # live columns - with the exact-diagonal tile at its
                        # head. One score/PV/den matmul per dg.
                        for dg in range(KPG):
                            ki = qi * KPG + dg
                            n = TQ - dg * TK
                            dsl = slice(dg * TK, TQ)
                            pex = qk_exp(ki, qi * TQ + dg * TK, n, True)
                            st_col = (qi == 0 and dg == 0)
                            sp_col = (dg == KPG - 1)
                            nc.tensor.matmul(
                                outp[:, dsl], vv[:, ki, hs], pex[:, 0:n],
                                start=st_col, stop=sp_col,
                                skip_group_check=True,
                            )
                            nc.tensor.matmul(
                                denp[:, dsl], ones_sb[:], pex[:, 0:n],
                                start=st_col, stop=sp_col,
                                skip_group_check=True,
                            )

                        def norm_and_project(h=h, qi=qi, outp=outp, denp=denp,
                                             b=b):
                            # deferred one q-group: runs while the PE chews
                            # on the next q-group, so the reciprocal chain
                            # never stalls the PE stream. 1/den computed as
                            # exp(-ln(den)) on the ACT engine: two table ops
                            # (~1e-3 rel err, fine for a softmax denominator)
                            # instead of the 13x-slower DVE reciprocal.
                            lnt = sax.tile([DH, TQ], f32, tag="lnt",
                                           name="lnt")
                            nc.scalar.activation(lnt[:], denp[:], LN)
                            rcp = sax.tile([DH, TQ], f32, tag="rcp",
                                           name="rcp")
                            nc.scalar.activation(rcp[:], lnt[:], EXP,
                                                 scale=-1.0)
                            otn = otnp.tile([DH, TQ], bf16, tag="otn",
                                            name="otn")
                            nc.vector.tensor_mul(otn[:], outp[:], rcp[:])
                            otn_tiles[(h, qi)] = otn
                            if h != HPC - 1:
                                return
                            for tc2 in range(TQ // TK):
                                tq0 = qi * TQ + tc2 * TK
                                for nch in range(d_sz // 512):
                                    yp = psY.tile([TK, 512], f32, tag="y",
                                                  name="yp")
                                    for hh in range(HPC):
                                        nc.tensor.matmul(
                                            yp[:],
                                            otn_tiles[(hh, qi)][
                                                :, tc2 * TK:(tc2 + 1) * TK],
                                            wo_sb[:, hh,
                                                  nch * 512:(nch + 1) * 512],
                                            start=(hh == 0),
                                            stop=(hh == HPC - 1),
                                        )
                                    # DVE only: the ACT engine stays free
                                    # for the latency-critical softmax exps
                                    ysb = pexp.tile([TK, 512], bf16, tag="ysb",
                                                    bufs=3, name="ysb")
                                    nc.vector.tensor_copy(ysb[:], yp[:])
                                    nc.sync.dma_start(
                                        y_r[b, tq0:tq0 + TK,
                                            nch * 512:(nch + 1) * 512],
                                        ysb[:])

                        pending.append(norm_and_project)
                        if len(pending) > 1:
                            pending.pop(0)()
                for fn in pending:
                    fn()
    if legalize:
        _legalize_waits(nc, mybir)
    return nc


_NC_CACHE = {}
LAST_RESULT = None


def _get_nc(b_sz, t_sz, d_sz):
    key = (b_sz, t_sz, d_sz)
    if key not in _NC_CACHE:
        _NC_CACHE[key] = _build_nc(b_sz, t_sz, d_sz)
    return _NC_CACHE[key]


def kernel(x, w_q, w_k, w_v, w_o):
    import ml_dtypes
    from concourse.bass_utils import run_bass_kernel_spmd

    bf16 = ml_dtypes.bfloat16
    b_sz, t_sz, d_sz = x.shape
    scale = np.float32(1.0 / np.sqrt(DH))

    xT = np.ascontiguousarray(
        np.asarray(x, np.float32).transpose(0, 2, 1)).astype(bf16)
    w_q = np.asarray(w_q, np.float32)
    w_k = np.asarray(w_k, np.float32)
    w_v = np.asarray(w_v, np.float32)
    w_o = np.asarray(w_o, np.float32)
    cosT, sinT = _rope_tables(t_sz, DH, THETA)
    r = np.arange(TK)
    tri01 = (r[None, :] >= r[:, None]).astype(bf16)  # [kv, q]: keep q >= kv

    in_maps = []
    for c in range(NCORES):
        cs = slice(c * HPC * DH, (c + 1) * HPC * DH)
        in_maps.append({
            "xT": xT,
            "wq": np.ascontiguousarray(w_q[:, cs] * scale).astype(bf16),
            "wk": np.ascontiguousarray(w_k[:, cs]).astype(bf16),
            "wv": np.ascontiguousarray(w_v[:, cs]).astype(bf16),
            "wo": np.ascontiguousarray(w_o[cs, :]).astype(bf16),
            "cos": cosT,
            "sin": sinT,
            "tri": tri01,
            "one": np.ones((128, 128), bf16),
        })

    nc = _get_nc(b_sz, t_sz, d_sz)
    res = run_bass_kernel_spmd(nc, in_maps, core_ids=list(range(NCORES)))
    global LAST_RESULT
    LAST_RESULT = res

    out = res.results[0]["y"].astype(np.float32)
    for c in range(1, NCORES):
        out += res.results[c]["y"].astype(np.float32)
    return out


# revision 26
# speedup vs baseline: 1.3532x; 1.0298x over previous
"""Causal self-attention with RoPE on 8 Trainium2 NeuronCores.

Sharding: Megatron-style head parallelism. 16 heads / 8 cores = 2 heads per
core. Each core computes q/k/v projections for its 2 heads (column-parallel),
full causal attention for those heads, and a partial output projection
(row-parallel slice of w_o). The host sums the 8 partial outputs.

v2 changes vs the f32r baseline:
- All matmul operands and all HBM traffic are bf16 (fp32 PSUM accumulate).
  Halves DMA bytes and SBUF read pressure; rel-err budget ~0.8% << 2e-2.
- Softmax denominators accumulate via an all-ones [128,128] lhsT, so the
  per-q sums land already replicated across all 128 partitions: the old
  [1,TQ] sum + ones-column broadcast matmul (which ran at 2 cyc/row) and
  the PSUM->SBUF staging copies are gone. The reciprocal runs directly on
  the PSUM tile via reciprocal_approx_fast (~5x faster than reciprocal),
  and the normalization multiply reads the PV PSUM tile directly.
- Fine-grained causal diagonal: the TQ x TQ diagonal square of each q-group
  is processed in 128-wide q-subchunks, only the lower-triangular kv tiles
  are computed, and the single exact-diagonal tile per subchunk is masked
  multiplicatively on the DVE after exp (zero the j<r triangle) instead of
  accumulating a -1e30 additive mask through the PE. Saves ~25% of the
  attention-phase PE rows.

On-chip layout: everything transposed. Host passes xT = x^T per batch
[B, D, T]; projections produce qT/kT [dh, t] directly and v [t, dh]
(lhsT = xT chunk, rhs = w_v slice). Scores are computed transposed,
ST[kv, q] = matmul(lhsT=kT_chunk, rhs=qT_group), which makes P^T directly
usable as the moving operand of the PV matmul - no on-chip transposes.
Normalization + output projection for each q group are emitted one q group
late so the PE stream never waits on the DVE reciprocal. No max-subtraction:
logits are q.k/sqrt(dh) with unit-ish variance, |logit| < ~8 << 88 (fp32 exp
overflow), identical math to the max-subtracted reference. The attention
scale 1/sqrt(dh) is folded into w_q on the host.
"""

import numpy as np

B, T, D = 4, 2048, 2048
H, DH = 16, 128
NCORES = 8
HPC = H // NCORES  # heads per core
THETA = 10000.0

TT = 512  # projection t-tile (moving dim of q/k projection matmuls)
TQ = 512  # attention q-group width
TK = 128  # kv tile (contraction chunk of PV / partition dim of ST)


def _rope_tables(seq_len, d_head, theta):
    # Matches reference.rope_cos_sin numerics, then transposes to [dh, t]
    # and folds the rotate-half sign into sin.
    inv_freq = 1.0 / (theta ** (np.arange(0, d_head, 2, dtype=np.float32) / d_head))
    t = np.arange(seq_len, dtype=np.float32)
    freqs = np.einsum("i,j->ij", t, inv_freq)
    emb = np.concatenate([freqs, freqs], axis=-1)  # [T, dh]
    cosT = np.ascontiguousarray(np.cos(emb).astype(np.float32).T)  # [dh, T]
    sinT = np.ascontiguousarray(np.sin(emb).astype(np.float32).T)
    sgn = np.ones((d_head, 1), np.float32)
    sgn[: d_head // 2] = -1.0
    return cosT, sinT * sgn


def _legalize_waits(nc, mybir):
    """Walrus on this toolchain refuses more than one embedded sync wait
    per engine instruction. Hoist extra waits into standalone
    EventSemaphore instructions on the same engine queue (the sequencer
    executes them in-stream before the instruction, same gating)."""
    n = 0
    for f in nc.m.functions:
        for bb in f.blocks:
            out = []
            for inst in bb.instructions:
                si = inst.sync_info
                if (si and si.on_wait and len(si.on_wait) > 1
                        and not isinstance(inst, mybir.InstEventSemaphore)):
                    for w in si.on_wait[:-1]:
                        out.append(mybir.InstEventSemaphore(
                            name=f"WH-{n}", engine=inst.engine,
                            sync_info=mybir.SyncInfo(
                                on_wait=[w], on_update=[])))
                        n += 1
                    inst.sync_info = mybir.SyncInfo(
                        on_wait=[si.on_wait[-1]],
                        on_update=list(si.on_update))
                out.append(inst)
            bb.instructions = out
    return n


def _build_nc(b_sz, t_sz, d_sz, legalize=True):
    import concourse.bass as bass
    import concourse.tile as tile
    from concourse import mybir

    f32 = mybir.dt.float32
    bf16 = mybir.dt.bfloat16
    EXP = mybir.ActivationFunctionType.Exp
    LN = mybir.ActivationFunctionType.Ln

    DC = d_sz // 128         # contraction chunks
    NQG = t_sz // TQ         # q groups per (batch, head)
    NKT = t_sz // TK         # kv tiles
    KPG = TQ // TK           # kv tiles per q group (diagonal span)

    nc = bass.Bass("TRN2", target_bir_lowering=False, debug=False,
                   enable_asserts=False, dynamic_dma_scratch_size=2048)

    NW = HPC * DH
    xT = nc.dram_tensor("xT", [b_sz, d_sz, t_sz], bf16, kind="ExternalInput")
    wq = nc.dram_tensor("wq", [d_sz, NW], bf16, kind="ExternalInput")
    wk = nc.dram_tensor("wk", [d_sz, NW], bf16, kind="ExternalInput")
    wv = nc.dram_tensor("wv", [d_sz, NW], bf16, kind="ExternalInput")
    wo = nc.dram_tensor("wo", [HPC * DH, d_sz], bf16, kind="ExternalInput")
    cos = nc.dram_tensor("cos", [DH, t_sz], f32, kind="ExternalInput")
    sin = nc.dram_tensor("sin", [DH, t_sz], f32, kind="ExternalInput")
    tri = nc.dram_tensor("tri", [TK, TK], bf16, kind="ExternalInput")
    one = nc.dram_tensor("one", [128, 128], bf16, kind="ExternalInput")
    y = nc.dram_tensor("y", [b_sz, t_sz, d_sz], bf16, kind="ExternalOutput")

    xT_r = xT.ap().rearrange("b (dc p) t -> b p dc t", p=128)
    wq_r = wq.ap().rearrange("(dc p) n -> p dc n", p=128)
    wk_r = wk.ap().rearrange("(dc p) n -> p dc n", p=128)
    wv_r = wv.ap().rearrange("(dc p) n -> p dc n", p=128)
    wo_r = wo.ap().rearrange("(h p) n -> p h n", p=128)
    y_r = y.ap()

    with tile.TileContext(nc) as tc:
        with (
            tc.tile_pool(name="consts", bufs=1) as consts,
            tc.tile_pool(name="wpool", bufs=1) as wpool,
            tc.tile_pool(name="qkv", bufs=1) as qkv,
            tc.tile_pool(name="xpool", bufs=3) as xpool,
            tc.tile_pool(name="rope", bufs=2) as rope,
            tc.tile_pool(name="pex", bufs=3) as pexp,
            tc.tile_pool(name="sax", bufs=2) as sax,
            tc.tile_pool(name="otn", bufs=8) as otnp,
            tc.tile_pool(name="psS", bufs=2, space="PSUM") as psS,
            tc.tile_pool(name="psO", bufs=2, space="PSUM") as psO,
            tc.tile_pool(name="psR", bufs=2, space="PSUM") as psR,
            tc.tile_pool(name="psY", bufs=2, space="PSUM") as psY,
        ):
            cos_sb = consts.tile([DH, t_sz], f32)
            sin_sb = consts.tile([DH, t_sz], f32)
            tri_sb = consts.tile([TK, TK], bf16)
            ones_sb = consts.tile([128, 128], bf16)

            wq_sb = wpool.tile([128, DC, NW], bf16)
            wk_sb = wpool.tile([128, DC, NW], bf16)
            wv_sb = wpool.tile([128, DC, NW], bf16)
            wo_sb = wpool.tile([128, HPC, d_sz], bf16)

            # DMA issue runs on two engine queues in parallel: the sync
            # engine paces the x-tile / y streams, the (otherwise idle)
            # gpsimd engine issues weights + constants, so the cold start
            # is not serialized on one sequencer's ~0.6us per descriptor.
            xt_first = xpool.tile([128, DC, TT], bf16, tag="xt",
                                  name="xt_first")
            for dc in range(DC):
                nc.sync.dma_start(xt_first[:, dc, :], xT_r[0, :, dc, 0:TT])
                nc.scalar.dma_start(wq_sb[:, dc, :], wq_r[:, dc, :])
            for dc in range(DC):
                nc.scalar.dma_start(wk_sb[:, dc, :], wk_r[:, dc, :])
            # cos/sin of the first tile gate the very first RoPE op
            nc.scalar.dma_start(cos_sb[:, 0:TT], cos.ap()[:, 0:TT])
            nc.scalar.dma_start(sin_sb[:, 0:TT], sin.ap()[:, 0:TT])
            for dc in range(DC):
                nc.scalar.dma_start(wv_sb[:, dc, :], wv_r[:, dc, :])

            def load_consts():
                # everything here is first needed in the attention phase
                # (t > ~90us): emitted after tile 1's x DMAs
                for i in range(1, t_sz // TT):
                    sl = slice(i * TT, (i + 1) * TT)
                    nc.scalar.dma_start(cos_sb[:, sl], cos.ap()[:, sl])
                    nc.scalar.dma_start(sin_sb[:, sl], sin.ap()[:, sl])
                nc.scalar.dma_start(tri_sb[:], tri.ap())
                nc.scalar.dma_start(ones_sb[:], one.ap())
                for hh in range(HPC):
                    for nch in range(d_sz // 512):
                        nsl = slice(nch * 512, (nch + 1) * 512)
                        nc.scalar.dma_start(wo_sb[:, hh, nsl],
                                            wo_r[:, hh, nsl])

            for b in range(b_sz):
                # ---------------- phase A: projections + RoPE ----------
                qT = [qkv.tile([DH, t_sz], bf16, tag=f"qT{h}", name=f"qT{h}")
                      for h in range(HPC)]
                kT = [qkv.tile([DH, t_sz], bf16, tag=f"kT{h}", name=f"kT{h}")
                      for h in range(HPC)]
                vv = qkv.tile([128, NKT, HPC * DH], bf16, tag="vv", name="vv")

                for tt in range(t_sz // TT):
                    tsl = slice(tt * TT, (tt + 1) * TT)
                    if b == 0 and tt == 0:
                        xt = xt_first
                    else:
                        xt = xpool.tile([128, DC, TT], bf16, tag="xt",
                                        name="xt")
                        for dc in range(0, DC, 2):
                            nc.sync.dma_start(xt[:, dc:dc + 2, :],
                                              xT_r[b, :, dc:dc + 2, tsl])
                    if b == 0 and tt == 1:
                        load_consts()

                    for h in range(HPC):
                        hs = slice(h * DH, (h + 1) * DH)
                        for dst, w_sb in ((qT[h], wq_sb), (kT[h], wk_sb)):
                            pp = psS.tile([128, TT], f32, tag="st")
                            for dc in range(DC):
                                nc.tensor.matmul(
                                    pp[:],
                                    w_sb[:, dc, hs],
                                    xt[:, dc, :],
                                    start=(dc == 0), stop=(dc == DC - 1),
                                )
                            # RoPE: dst = pp*cos + swap(pp)*sin_signed
                            sh = rope.tile([DH, TT], f32, tag="sh")
                            nc.vector.tensor_mul(
                                sh[0:64, :], pp[64:128, :], sin_sb[0:64, tsl])
                            nc.vector.tensor_mul(
                                sh[64:128, :], pp[0:64, :], sin_sb[64:128, tsl])
                            tmp = rope.tile([DH, TT], f32, tag="tmp")
                            nc.vector.tensor_mul(tmp[:], pp[:], cos_sb[:, tsl])
                            nc.vector.tensor_add(dst[:, tsl], tmp[:], sh[:])

                    for ts2 in range(TT // TK):
                        ts3 = slice(ts2 * TK, (ts2 + 1) * TK)
                        vp = psS.tile([128, TT], f32, tag="st")
                        for dc in range(DC):
                            nc.tensor.matmul(
                                vp[:, 0:HPC * DH],
                                xt[:, dc, ts3],
                                wv_sb[:, dc, :],
                                start=(dc == 0), stop=(dc == DC - 1),
                            )
                        kv_i = tt * (TT // TK) + ts2
                        nc.scalar.copy(vv[:, kv_i, :], vp[:, 0:HPC * DH])

                # ---------------- phase B + C: attention + out proj ----
                otn_tiles = {}
                pending = []
                for h in range(HPC):
                    hs = slice(h * DH, (h + 1) * DH)
                    for qi in range(NQG):
                        outp = psO.tile([DH, TQ], f32, tag="outT")
                        denp = psR.tile([DH, TQ], f32, tag="den")

                        def qk_exp(ki, q0, n, masked):
                            # score matmul [TK, n] + exp (+ causal mask on
                            # the leading TK columns = the exact-diagonal
                            # tile, zeroed multiplicatively after exp)
                            stp = psS.tile([128, TT], f32, tag="st")
                            nc.tensor.matmul(
                                stp[:, 0:n],
                                kT[h][:, ki * TK:(ki + 1) * TK],
                                qT[h][:, q0:q0 + n],
                                start=True, stop=True,
                            )
                            pex = pexp.tile([TK, TQ], bf16, tag="pex",
                                            name="pex")
                            nc.scalar.activation(pex[:, 0:n], stp[:, 0:n],
                                                 EXP)
                            if masked:
                                nc.vector.tensor_mul(
                                    pex[:, 0:TK], pex[:, 0:TK], tri_sb[:])
                            return pex

                        # off-diagonal: full-width, no masking
                        nko = qi * KPG
                        for ki in range(nko):
                            pex = qk_exp(ki, qi * TQ, TQ, False)
                            nc.tensor.matmul(
                                outp[:], vv[:, ki, hs], pex[:],
                                start=(ki == 0), stop=False,
                                skip_group_check=True,
                            )
                            nc.tensor.matmul(
                                denp[:], ones_sb[:], pex[:],
                                start=(ki == 0), stop=False,
                                skip_group_check=True,
                            )
                        # diagonal square: kv tile dg covers the contiguous
                        # q-range [dg*TK, TQ) of this group - the causally
                        # live columns - with the exact-diagonal tile at its
                        # head. One score/PV/den matmul per dg.
                        for dg in range(KPG):
                            ki = qi * KPG + dg
                            n = TQ - dg * TK
                            dsl = slice(dg * TK, TQ)
                            pex = qk_exp(ki, qi * TQ + dg * TK, n, True)
                            st_col = (qi == 0 and dg == 0)
                            sp_col = (dg == KPG - 1)
                            nc.tensor.matmul(
                                outp[:, dsl], vv[:, ki, hs], pex[:, 0:n],
                                start=st_col, stop=sp_col,
                                skip_group_check=True,
                            )
                            nc.tensor.matmul(
                                denp[:, dsl], ones_sb[:], pex[:, 0:n],
                                start=st_col, stop=sp_col,
                                skip_group_check=True,
                            )

                        def norm_and_project(h=h, qi=qi, outp=outp, denp=denp,
                                             b=b):
                            # deferred one q-group: runs while the PE chews
                            # on the next q-group, so the reciprocal chain
                            # never stalls the PE stream. 1/den computed as
                            # exp(-ln(den)) on the ACT engine: two table ops
                            # (~1e-3 rel err, fine for a softmax denominator)
                            # instead of the 13x-slower DVE reciprocal.
                            lnt = sax.tile([DH, TQ], f32, tag="lnt",
                                           name="lnt")
                            nc.scalar.activation(lnt[:], denp[:], LN)
                            rcp = sax.tile([DH, TQ], f32, tag="rcp",
                                           name="rcp")
                            nc.scalar.activation(rcp[:], lnt[:], EXP,
                                                 scale=-1.0)
                            otn = otnp.tile([DH, TQ], bf16, tag="otn",
                                            name="otn")
                            nc.vector.tensor_mul(otn[:], outp[:], rcp[:])
                            otn_tiles[(h, qi)] = otn
                            if h != HPC - 1:
                                return
                            for tc2 in range(TQ // TK):
                                tq0 = qi * TQ + tc2 * TK
                                # one wide staging tile per 128-token row
                                # block: 2 half-row DMAs (2KB descriptors)
                                # instead of 4, keeping the sync queue free
                                # to prefetch the next batch's x tiles
                                ysb = pexp.tile([TK, d_sz], bf16, tag="ysb",
                                                bufs=4, name="ysb")
                                for nch in range(d_sz // 512):
                                    yp = psY.tile([TK, 512], f32, tag="y",
                                                  name="yp")
                                    for hh in range(HPC):
                                        nc.tensor.matmul(
                                            yp[:],
                                            otn_tiles[(hh, qi)][
                                                :, tc2 * TK:(tc2 + 1) * TK],
                                            wo_sb[:, hh,
                                                  nch * 512:(nch + 1) * 512],
                                            start=(hh == 0),
                                            stop=(hh == HPC - 1),
                                        )
                                    # DVE only: the ACT engine stays free
                                    # for the latency-critical softmax exps
                                    nc.vector.tensor_copy(
                                        ysb[:, nch * 512:(nch + 1) * 512],
                                        yp[:])
                                for half in range(2):
                                    hsl = slice(half * (d_sz // 2),
                                                (half + 1) * (d_sz // 2))
                                    nc.sync.dma_start(
                                        y_r[b, tq0:tq0 + TK, hsl],
                                        ysb[:, hsl])

                        pending.append(norm_and_project)
                        if len(pending) > 1:
                            pending.pop(0)()
                for fn in pending:
                    fn()
    if legalize:
        _legalize_waits(nc, mybir)
    return nc


_NC_CACHE = {}
LAST_RESULT = None


def _get_nc(b_sz, t_sz, d_sz):
    key = (b_sz, t_sz, d_sz)
    if key not in _NC_CACHE:
        _NC_CACHE[key] = _build_nc(b_sz, t_sz, d_sz)
    return _NC_CACHE[key]


def kernel(x, w_q, w_k, w_v, w_o):
    import ml_dtypes
    from concourse.bass_utils import run_bass_kernel_spmd

    bf16 = ml_dtypes.bfloat16
    b_sz, t_sz, d_sz = x.shape
    scale = np.float32(1.0 / np.sqrt(DH))

    xT = np.ascontiguousarray(
        np.asarray(x, np.float32).transpose(0, 2, 1)).astype(bf16)
    w_q = np.asarray(w_q, np.float32)
    w_k = np.asarray(w_k, np.float32)
    w_v = np.asarray(w_v, np.float32)
    w_o = np.asarray(w_o, np.float32)
    cosT, sinT = _rope_tables(t_sz, DH, THETA)
    r = np.arange(TK)
    tri01 = (r[None, :] >= r[:, None]).astype(bf16)  # [kv, q]: keep q >= kv

    in_maps = []
    for c in range(NCORES):
        cs = slice(c * HPC * DH, (c + 1) * HPC * DH)
        in_maps.append({
            "xT": xT,
            "wq": np.ascontiguousarray(w_q[:, cs] * scale).astype(bf16),
            "wk": np.ascontiguousarray(w_k[:, cs]).astype(bf16),
            "wv": np.ascontiguousarray(w_v[:, cs]).astype(bf16),
            "wo": np.ascontiguousarray(w_o[cs, :]).astype(bf16),
            "cos": cosT,
            "sin": sinT,
            "tri": tri01,
            "one": np.ones((128, 128), bf16),
        })

    nc = _get_nc(b_sz, t_sz, d_sz)
    res = run_bass_kernel_spmd(nc, in_maps, core_ids=list(range(NCORES)))
    global LAST_RESULT
    LAST_RESULT = res

    out = res.results[0]["y"].astype(np.float32)
    for c in range(1, NCORES):
        out += res.results[c]["y"].astype(np.float32)
    return out


# revision 30
# speedup vs baseline: 1.3923x; 1.0289x over previous
"""Causal self-attention with RoPE on 8 Trainium2 NeuronCores.

Sharding: Megatron-style head parallelism. 16 heads / 8 cores = 2 heads per
core. Each core computes q/k/v projections for its 2 heads (column-parallel),
full causal attention for those heads, and a partial output projection
(row-parallel slice of w_o). The host sums the 8 partial outputs.

v2 changes vs the f32r baseline:
- All matmul operands and all HBM traffic are bf16 (fp32 PSUM accumulate).
  Halves DMA bytes and SBUF read pressure; rel-err budget ~0.8% << 2e-2.
- Softmax denominators accumulate via an all-ones [128,128] lhsT, so the
  per-q sums land already replicated across all 128 partitions: the old
  [1,TQ] sum + ones-column broadcast matmul (which ran at 2 cyc/row) and
  the PSUM->SBUF staging copies are gone. The reciprocal runs directly on
  the PSUM tile via reciprocal_approx_fast (~5x faster than reciprocal),
  and the normalization multiply reads the PV PSUM tile directly.
- Fine-grained causal diagonal: the TQ x TQ diagonal square of each q-group
  is processed in 128-wide q-subchunks, only the lower-triangular kv tiles
  are computed, and the single exact-diagonal tile per subchunk is masked
  multiplicatively on the DVE after exp (zero the j<r triangle) instead of
  accumulating a -1e30 additive mask through the PE. Saves ~25% of the
  attention-phase PE rows.

On-chip layout: everything transposed. Host passes xT = x^T per batch
[B, D, T]; projections produce qT/kT [dh, t] directly and v [t, dh]
(lhsT = xT chunk, rhs = w_v slice). Scores are computed transposed,
ST[kv, q] = matmul(lhsT=kT_chunk, rhs=qT_group), which makes P^T directly
usable as the moving operand of the PV matmul - no on-chip transposes.
Normalization + output projection for each q group are emitted one q group
late so the PE stream never waits on the DVE reciprocal. No max-subtraction:
logits are q.k/sqrt(dh) with unit-ish variance, |logit| < ~8 << 88 (fp32 exp
overflow), identical math to the max-subtracted reference. The attention
scale 1/sqrt(dh) is folded into w_q on the host.
"""

import numpy as np

B, T, D = 4, 2048, 2048
H, DH = 16, 128
NCORES = 8
HPC = H // NCORES  # heads per core
THETA = 10000.0

TT = 512  # projection t-tile (moving dim of q/k projection matmuls)
TQ = 512  # attention q-group width
TK = 128  # kv tile (contraction chunk of PV / partition dim of ST)


def _rope_tables(seq_len, d_head, theta):
    # Matches reference.rope_cos_sin numerics, then transposes to [dh, t]
    # and folds the rotate-half sign into sin.
    inv_freq = 1.0 / (theta ** (np.arange(0, d_head, 2, dtype=np.float32) / d_head))
    t = np.arange(seq_len, dtype=np.float32)
    freqs = np.einsum("i,j->ij", t, inv_freq)
    emb = np.concatenate([freqs, freqs], axis=-1)  # [T, dh]
    cosT = np.ascontiguousarray(np.cos(emb).astype(np.float32).T)  # [dh, T]
    sinT = np.ascontiguousarray(np.sin(emb).astype(np.float32).T)
    sgn = np.ones((d_head, 1), np.float32)
    sgn[: d_head // 2] = -1.0
    return cosT, sinT * sgn


def _legalize_waits(nc, mybir):
    """Walrus on this toolchain refuses more than one embedded sync wait
    per engine instruction. Hoist extra waits into standalone
    EventSemaphore instructions on the same engine queue (the sequencer
    executes them in-stream before the instruction, same gating)."""
    n = 0
    for f in nc.m.functions:
        for bb in f.blocks:
            out = []
            for inst in bb.instructions:
                si = inst.sync_info
                if (si and si.on_wait and len(si.on_wait) > 1
                        and not isinstance(inst, mybir.InstEventSemaphore)):
                    for w in si.on_wait[:-1]:
                        out.append(mybir.InstEventSemaphore(
                            name=f"WH-{n}", engine=inst.engine,
                            sync_info=mybir.SyncInfo(
                                on_wait=[w], on_update=[])))
                        n += 1
                    inst.sync_info = mybir.SyncInfo(
                        on_wait=[si.on_wait[-1]],
                        on_update=list(si.on_update))
                out.append(inst)
            bb.instructions = out
    return n


def _build_nc(b_sz, t_sz, d_sz, legalize=True):
    import concourse.bass as bass
    import concourse.tile as tile
    from concourse import mybir

    f32 = mybir.dt.float32
    bf16 = mybir.dt.bfloat16
    EXP = mybir.ActivationFunctionType.Exp
    LN = mybir.ActivationFunctionType.Ln

    DC = d_sz // 128         # contraction chunks
    NQG = t_sz // TQ         # q groups per (batch, head)
    NKT = t_sz // TK         # kv tiles
    KPG = TQ // TK           # kv tiles per q group (diagonal span)

    nc = bass.Bass("TRN2", target_bir_lowering=False, debug=False,
                   enable_asserts=False, dynamic_dma_scratch_size=2048)

    NW = HPC * DH
    xT = nc.dram_tensor("xT", [b_sz, d_sz, t_sz], bf16, kind="ExternalInput")
    wq = nc.dram_tensor("wq", [d_sz, NW], bf16, kind="ExternalInput")
    wk = nc.dram_tensor("wk", [d_sz, NW], bf16, kind="ExternalInput")
    wv = nc.dram_tensor("wv", [d_sz, NW], bf16, kind="ExternalInput")
    wo = nc.dram_tensor("wo", [HPC * DH, d_sz], bf16, kind="ExternalInput")
    cos = nc.dram_tensor("cos", [DH, t_sz], f32, kind="ExternalInput")
    sin = nc.dram_tensor("sin", [DH, t_sz], f32, kind="ExternalInput")
    tri = nc.dram_tensor("tri", [TK, TK], bf16, kind="ExternalInput")
    one = nc.dram_tensor("one", [128, 128], bf16, kind="ExternalInput")
    y = nc.dram_tensor("y", [b_sz, t_sz, d_sz], bf16, kind="ExternalOutput")

    xT_r = xT.ap().rearrange("b (dc p) t -> b p dc t", p=128)
    wq_r = wq.ap().rearrange("(dc p) n -> p dc n", p=128)
    wk_r = wk.ap().rearrange("(dc p) n -> p dc n", p=128)
    wv_r = wv.ap().rearrange("(dc p) n -> p dc n", p=128)
    wo_r = wo.ap().rearrange("(h p) n -> p h n", p=128)
    y_r = y.ap()

    with tile.TileContext(nc) as tc:
        with (
            tc.tile_pool(name="consts", bufs=1) as consts,
            tc.tile_pool(name="wpool", bufs=1) as wpool,
            tc.tile_pool(name="qkv", bufs=1) as qkv,
            tc.tile_pool(name="xpool", bufs=3) as xpool,
            tc.tile_pool(name="rope", bufs=2) as rope,
            tc.tile_pool(name="pex", bufs=3) as pexp,
            tc.tile_pool(name="sax", bufs=2) as sax,
            tc.tile_pool(name="otn", bufs=10) as otnp,
            tc.tile_pool(name="psS", bufs=2, space="PSUM") as psS,
            tc.tile_pool(name="psO", bufs=2, space="PSUM") as psO,
            tc.tile_pool(name="psR", bufs=2, space="PSUM") as psR,
            tc.tile_pool(name="psY", bufs=2, space="PSUM") as psY,
        ):
            cos_sb = consts.tile([DH, t_sz], f32)
            sin_sb = consts.tile([DH, t_sz], f32)
            tri_sb = consts.tile([TK, TK], bf16)
            ones_sb = consts.tile([128, 128], bf16)

            wq_sb = wpool.tile([128, DC, NW], bf16)
            wk_sb = wpool.tile([128, DC, NW], bf16)
            wv_sb = wpool.tile([128, DC, NW], bf16)
            wo_sb = wpool.tile([128, HPC, d_sz], bf16)

            # DMA issue runs on two engine queues in parallel: the sync
            # engine paces the x-tile / y streams, the (otherwise idle)
            # gpsimd engine issues weights + constants, so the cold start
            # is not serialized on one sequencer's ~0.6us per descriptor.
            xt_first = xpool.tile([128, DC, TT], bf16, tag="xt",
                                  name="xt_first")
            for dc in range(DC):
                nc.sync.dma_start(xt_first[:, dc, :], xT_r[0, :, dc, 0:TT])
                nc.scalar.dma_start(wq_sb[:, dc, :], wq_r[:, dc, :])
            for dc in range(DC):
                nc.scalar.dma_start(wk_sb[:, dc, :], wk_r[:, dc, :])
            # cos/sin of the first tile gate the very first RoPE op
            nc.scalar.dma_start(cos_sb[:, 0:TT], cos.ap()[:, 0:TT])
            nc.scalar.dma_start(sin_sb[:, 0:TT], sin.ap()[:, 0:TT])
            for dc in range(DC):
                nc.scalar.dma_start(wv_sb[:, dc, :], wv_r[:, dc, :])

            def load_consts():
                # everything here is first needed in the attention phase
                # (t > ~90us): emitted after tile 1's x DMAs
                for i in range(1, t_sz // TT):
                    sl = slice(i * TT, (i + 1) * TT)
                    nc.scalar.dma_start(cos_sb[:, sl], cos.ap()[:, sl])
                    nc.scalar.dma_start(sin_sb[:, sl], sin.ap()[:, sl])
                nc.scalar.dma_start(tri_sb[:], tri.ap())
                nc.scalar.dma_start(ones_sb[:], one.ap())
                for hh in range(HPC):
                    for nch in range(d_sz // 512):
                        nsl = slice(nch * 512, (nch + 1) * 512)
                        nc.scalar.dma_start(wo_sb[:, hh, nsl],
                                            wo_r[:, hh, nsl])

            # deferred-normalization / output-projection closures carry
            # across batch boundaries so no per-batch pipeline drain
            otn_tiles = {}
            pending1 = []
            pending2 = []
            for b in range(b_sz):
                # ---------------- phase A: projections + RoPE ----------
                qT = [qkv.tile([DH, t_sz], bf16, tag=f"qT{h}", name=f"qT{h}")
                      for h in range(HPC)]
                kT = [qkv.tile([DH, t_sz], bf16, tag=f"kT{h}", name=f"kT{h}")
                      for h in range(HPC)]
                vv = qkv.tile([128, NKT, HPC * DH], bf16, tag="vv", name="vv")

                for tt in range(t_sz // TT):
                    tsl = slice(tt * TT, (tt + 1) * TT)
                    if b == 0 and tt == 0:
                        xt = xt_first
                    else:
                        xt = xpool.tile([128, DC, TT], bf16, tag="xt",
                                        name="xt")
                        for dc in range(0, DC, 2):
                            nc.sync.dma_start(xt[:, dc:dc + 2, :],
                                              xT_r[b, :, dc:dc + 2, tsl])
                    if b == 0 and tt == 1:
                        load_consts()

                    for h in range(HPC):
                        hs = slice(h * DH, (h + 1) * DH)
                        for dst, w_sb in ((qT[h], wq_sb), (kT[h], wk_sb)):
                            pp = psS.tile([128, TT], f32, tag="st")
                            for dc in range(DC):
                                nc.tensor.matmul(
                                    pp[:],
                                    w_sb[:, dc, hs],
                                    xt[:, dc, :],
                                    start=(dc == 0), stop=(dc == DC - 1),
                                )
                            # RoPE: dst = pp*cos + swap(pp)*sin_signed
                            sh = rope.tile([DH, TT], f32, tag="sh")
                            nc.vector.tensor_mul(
                                sh[0:64, :], pp[64:128, :], sin_sb[0:64, tsl])
                            nc.vector.tensor_mul(
                                sh[64:128, :], pp[0:64, :], sin_sb[64:128, tsl])
                            tmp = rope.tile([DH, TT], f32, tag="tmp")
                            nc.vector.tensor_mul(tmp[:], pp[:], cos_sb[:, tsl])
                            nc.vector.tensor_add(dst[:, tsl], tmp[:], sh[:])

                    for ts2 in range(TT // TK):
                        ts3 = slice(ts2 * TK, (ts2 + 1) * TK)
                        vp = psS.tile([128, TT], f32, tag="st")
                        for dc in range(DC):
                            nc.tensor.matmul(
                                vp[:, 0:HPC * DH],
                                xt[:, dc, ts3],
                                wv_sb[:, dc, :],
                                start=(dc == 0), stop=(dc == DC - 1),
                            )
                        kv_i = tt * (TT // TK) + ts2
                        nc.scalar.copy(vv[:, kv_i, :], vp[:, 0:HPC * DH])

                # ---------------- phase B + C: attention + out proj ----
                for h in range(HPC):
                    hs = slice(h * DH, (h + 1) * DH)
                    for qi in range(NQG):
                        outp = psO.tile([DH, TQ], f32, tag="outT")
                        denp = psR.tile([DH, TQ], f32, tag="den")

                        def qk_exp(ki, q0, n, masked):
                            # score matmul [TK, n] + exp (+ causal mask on
                            # the leading TK columns = the exact-diagonal
                            # tile, zeroed multiplicatively after exp)
                            stp = psS.tile([128, TT], f32, tag="st")
                            nc.tensor.matmul(
                                stp[:, 0:n],
                                kT[h][:, ki * TK:(ki + 1) * TK],
                                qT[h][:, q0:q0 + n],
                                start=True, stop=True,
                            )
                            pex = pexp.tile([TK, TQ], bf16, tag="pex",
                                            name="pex")
                            nc.scalar.activation(pex[:, 0:n], stp[:, 0:n],
                                                 EXP)
                            if masked:
                                nc.vector.tensor_mul(
                                    pex[:, 0:TK], pex[:, 0:TK], tri_sb[:])
                            return pex

                        # kv tiles of this q group: off-diagonal full-width
                        # tiles, then the diagonal tiles, each covering the
                        # causally live q-range [dg*TK, TQ). The PV/den
                        # matmuls for tile i are emitted AFTER the score of
                        # tile i+1, so each exp hides under the next score
                        # and the PE never waits on the ACT engine.
                        items = [(ki, 0, TQ, False, ki == 0, False)
                                 for ki in range(qi * KPG)]
                        items += [(qi * KPG + dg, dg * TK, TQ - dg * TK,
                                   True, qi == 0 and dg == 0, dg == KPG - 1)
                                  for dg in range(KPG)]

                        def emit_pv(it, pex):
                            ki, c0, n, _, st_f, sp_f = it
                            dsl = slice(c0, c0 + n)
                            nc.tensor.matmul(
                                outp[:, dsl], vv[:, ki, hs], pex[:, 0:n],
                                start=st_f, stop=sp_f,
                                skip_group_check=True,
                            )
                            nc.tensor.matmul(
                                denp[:, dsl], ones_sb[:], pex[:, 0:n],
                                start=st_f, stop=sp_f,
                                skip_group_check=True,
                            )

                        pend = None
                        for it in items:
                            pex = qk_exp(it[0], qi * TQ + it[1], it[2],
                                         it[3])
                            if pend is not None:
                                emit_pv(*pend)
                            pend = (it, pex)
                        emit_pv(*pend)

                        def stage1(h=h, qi=qi, outp=outp, denp=denp):
                            # deferred one q-group: 1/den computed as
                            # exp(-ln(den)) on the ACT engine: two table ops
                            # (~1e-3 rel err, fine for a softmax
                            # denominator) instead of the 13x-slower DVE
                            # reciprocal; the normalization multiply reads
                            # the PV PSUM tile directly.
                            lnt = sax.tile([DH, TQ], f32, tag="lnt",
                                           name="lnt")
                            nc.scalar.activation(lnt[:], denp[:], LN)
                            rcp = sax.tile([DH, TQ], f32, tag="rcp",
                                           name="rcp")
                            nc.scalar.activation(rcp[:], lnt[:], EXP,
                                                 scale=-1.0)
                            otn = otnp.tile([DH, TQ], bf16, tag="otn",
                                            name="otn")
                            nc.vector.tensor_mul(otn[:], outp[:], rcp[:])
                            otn_tiles[(h, qi)] = otn

                        def stage2(qi=qi, b=b):
                            # deferred two q-groups: by now the otn tiles of
                            # both heads exist and their DVE writes have had
                            # a full group to drain
                            for tc2 in range(TQ // TK):
                                tq0 = qi * TQ + tc2 * TK
                                # one wide staging tile per 128-token row
                                # block: 2 half-row DMAs (2KB descriptors)
                                # instead of 4, keeping the sync queue free
                                # to prefetch the next batch's x tiles
                                ysb = pexp.tile([TK, d_sz], bf16, tag="ysb",
                                                bufs=4, name="ysb")
                                for nch in range(d_sz // 512):
                                    yp = psY.tile([TK, 512], f32, tag="y",
                                                  name="yp")
                                    for hh in range(HPC):
                                        nc.tensor.matmul(
                                            yp[:],
                                            otn_tiles[(hh, qi)][
                                                :, tc2 * TK:(tc2 + 1) * TK],
                                            wo_sb[:, hh,
                                                  nch * 512:(nch + 1) * 512],
                                            start=(hh == 0),
                                            stop=(hh == HPC - 1),
                                        )
                                    # DVE only: the ACT engine stays free
                                    # for the latency-critical softmax exps
                                    nc.vector.tensor_copy(
                                        ysb[:, nch * 512:(nch + 1) * 512],
                                        yp[:])
                                for half in range(2):
                                    hsl = slice(half * (d_sz // 2),
                                                (half + 1) * (d_sz // 2))
                                    nc.sync.dma_start(
                                        y_r[b, tq0:tq0 + TK, hsl],
                                        ysb[:, hsl])

                        pending1.append(stage1)
                        pending2.append(stage2 if h == HPC - 1 else None)
                        if len(pending1) > 1:
                            pending1.pop(0)()
                        if len(pending2) > 2:
                            fn = pending2.pop(0)
                            if fn is not None:
                                fn()
            for fn in pending1:
                fn()
            for fn in pending2:
                if fn is not None:
                    fn()
    if legalize:
        _legalize_waits(nc, mybir)
    return nc


_NC_CACHE = {}
LAST_RESULT = None


def _get_nc(b_sz, t_sz, d_sz):
    key = (b_sz, t_sz, d_sz)
    if key not in _NC_CACHE:
        _NC_CACHE[key] = _build_nc(b_sz, t_sz, d_sz)
    return _NC_CACHE[key]


def kernel(x, w_q, w_k, w_v, w_o):
    import ml_dtypes
    from concourse.bass_utils import run_bass_kernel_spmd

    bf16 = ml_dtypes.bfloat16
    b_sz, t_sz, d_sz = x.shape
    scale = np.float32(1.0 / np.sqrt(DH))

    xT = np.ascontiguousarray(
        np.asarray(x, np.float32).transpose(0, 2, 1)).astype(bf16)
    w_q = np.asarray(w_q, np.float32)
    w_k = np.asarray(w_k, np.float32)
    w_v = np.asarray(w_v, np.float32)
    w_o = np.asarray(w_o, np.float32)
    cosT, sinT = _rope_tables(t_sz, DH, THETA)
    r = np.arange(TK)
    tri01 = (r[None, :] >= r[:, None]).astype(bf16)  # [kv, q]: keep q >= kv

    in_maps = []
    for c in range(NCORES):
        cs = slice(c * HPC * DH, (c + 1) * HPC * DH)
        in_maps.append({
            "xT": xT,
            "wq": np.ascontiguousarray(w_q[:, cs] * scale).astype(bf16),
            "wk": np.ascontiguousarray(w_k[:, cs]).astype(bf16),
            "wv": np.ascontiguousarray(w_v[:, cs]).astype(bf16),
            "wo": np.ascontiguousarray(w_o[cs, :]).astype(bf16),
            "cos": cosT,
            "sin": sinT,
            "tri": tri01,
            "one": np.ones((128, 128), bf16),
        })

    nc = _get_nc(b_sz, t_sz, d_sz)
    res = run_bass_kernel_spmd(nc, in_maps, core_ids=list(range(NCORES)))
    global LAST_RESULT
    LAST_RESULT = res

    out = res.results[0]["y"].astype(np.float32)
    for c in range(1, NCORES):
        out += res.results[c]["y"].astype(np.float32)
    return out
